# revision 1
# baseline (speedup 1.0000x reference)
"""Trainium2 Bass kernel for nn_EnhancedObj (gnn_message_passing).

Per batch sample (data-parallel over 8 cores, one sample per core):
    ve  = LN(tanh(visual @ W_v + b_v))                  [64, 2048]
    oe  = LN(tanh(obj_flat @ W_o + b_o))                [2304, 2048]
    adj = softmax_n(oe @ ve^T / sqrt(2048))             [2304, 64]
    out = LN(tanh(adj^T @ oe + ve))                     [64, 2048]

All matmuls run in fp16 (fp32 PSUM accumulate).  Softmax and all
LayerNorm statistics are fp32.

Schedule: ONE fused PE stream.  Chunks 0-1 run kc-outer across both
chunks (8 PSUM banks) so the PE consumes each W_o k-slice the moment
it lands instead of stalling on the serial W_o stream.  The visual
branch (A) is emitted between object chunks 3 and 4, consuming W_v
that streamed in behind W_o; the adjacency (C) and aggregation (D)
matmuls interleave into the stream two chunks at a time, with oe
transposes riding the sync HWDGE queue behind the weight streams.
Softmax uses unnormalized exp weights (logits are O(1)-bounded, so no
max subtraction); the aggregation is rescaled by the global 1/sum at
the end.

LayerNorm's 1/sqrt(var+eps) is computed ON THE VECTOR ENGINE with a
bit-hack seed + 2 Newton steps (~5e-6 rel err).  This keeps the scalar
engine exclusively on Tanh/Exp, which share one activation table —
the baseline's 40 x 1.28us ACT_TABLE_LOAD swaps (Sqrt lives in a
different table) are eliminated entirely, unblocking the in-order
scalar queue that recycles PSUM banks.

The endgame splits the final window's exp into halves so the en
transposes overlap D matmuls, and the last D window completes
per-quarter so the final rescale/tanh/LN pipeline overlaps the PE
drain.

The device kernel assumes the spec's deterministic fills (zero biases,
unit gains).  If non-trivial bias/gain vectors are ever passed, we
fall back to an exact fp32 numpy implementation.
"""

import numpy as np

F16 = np.float16

BS = 8          # batch (== number of cores)
F = 64          # win_len (frames)
OBJ = 36        # objects per frame
D = 2048        # feature dim
N = F * OBJ     # 2304 objects per sample
NCH = N // 128  # 18 object-row chunks
NW = NCH // 2   # 9 two-chunk adjacency windows
KC = D // 128   # 16 contraction chunks
DW = 512        # matmul moving width (one PSUM bank of fp32)
ND = D // DW    # 4 output-column groups
LN_EPS = 1e-5
RSQRT_MAGIC = 0x5F3759DF

_BUILD_CACHE = {}


def _f32(x):
    return np.ascontiguousarray(np.asarray(x), dtype=np.float32)


def _klc_layout(w):
    """[D, M] -> [128(kl), KC*M] with element (kl, kc, m) = w[kc*128+kl, m]."""
    d, m = w.shape
    assert d == D
    return w.reshape(KC, 128, m).transpose(1, 0, 2).reshape(128, KC * m)


def _build():
    """Build + compile the SPMD Bass program (trivial-fill fast path)."""
    if "nc" in _BUILD_CACHE:
        return _BUILD_CACHE["nc"]

    import concourse.bacc as bacc
    import concourse.tile as tile
    from concourse import mybir

    f32 = mybir.dt.float32
    f16 = mybir.dt.float16
    i32 = mybir.dt.int32
    AF = mybir.ActivationFunctionType
    AX = mybir.AxisListType
    OP = mybir.AluOpType

    nc = bacc.Bacc("TRN2", target_bir_lowering=False, debug=False, num_devices=BS)

    objT_d = nc.dram_tensor("objT", [NCH, 128, KC * 128], f16, kind="ExternalInput").ap()
    wo_d = nc.dram_tensor("Wo", [128, KC * D], f16, kind="ExternalInput").ap()
    wv_d = nc.dram_tensor("Wv", [128, KC * D], f16, kind="ExternalInput").ap()
    vt_d = nc.dram_tensor("vT", [128, KC * F], f16, kind="ExternalInput").ap()
    out_d = nc.dram_tensor("out", [F, D], f32, kind="ExternalOutput").ap()

    inv_sqrt_d = 1.0 / float(np.sqrt(D))

    # adjacency (C) / aggregation (D) emission points: window w covers
    # object chunks (2w, 2w+1); C(w) needs both transposed + veT (ready
    # after chunk 4); D(w) follows C(w) one chunk later.  Window NW-1
    # drains manually after the loop.
    sched = {}
    for w in range(NW):
        c_at = max(2 * w + 3, 5 + (0 if w < 3 else 0)) if w >= 3 else 5 + w
        c_at = min(c_at, NCH - 1) if w < NW - 1 else NCH  # NCH == post-loop
        d_at = c_at + 1
        if c_at < NCH:
            sched.setdefault(c_at, []).append(("C", w))
        if d_at < NCH:
            sched.setdefault(d_at, []).append(("D", w))

    with tile.TileContext(nc) as tc:
        with tc.tile_pool(name="persist", bufs=1) as persist, \
             tc.tile_pool(name="stats", bufs=2) as stats_pool:

            def ln_rsqrt(mvr, rows):
                """mvr[:,1]=var -> mvr[:,2]=1/sqrt(var+eps), vector engine
                only (bit-hack seed + 1 Newton step, ~1e-3 worst-case rel
                err on the LN scale; no act-table load)."""
                x, y, t = mvr[:rows, 3:4], mvr[:rows, 2:3], mvr[:rows, 4:5]
                nc.vector.tensor_scalar(out=x, in0=mvr[:rows, 1:2],
                                        scalar1=LN_EPS, scalar2=None, op0=OP.add)
                nc.vector.tensor_scalar(out=t.bitcast(i32), in0=x.bitcast(i32),
                                        scalar1=1, scalar2=None,
                                        op0=OP.logical_shift_right)
                # magic - (i>>1)  ==  (i>>1) * -1 + magic  (both ops arith)
                nc.vector.tensor_scalar(out=y.bitcast(i32), in0=t.bitcast(i32),
                                        scalar1=-1, scalar2=RSQRT_MAGIC,
                                        op0=OP.mult, op1=OP.add)
                for _ in range(1):
                    nc.vector.tensor_tensor(out=t, in0=y, in1=y, op=OP.mult)
                    nc.vector.tensor_tensor(out=t, in0=t, in1=x, op=OP.mult)
                    nc.vector.tensor_scalar(out=t, in0=t, scalar1=-0.5,
                                            scalar2=1.5, op0=OP.mult, op1=OP.add)
                    nc.vector.tensor_tensor(out=y, in0=y, in1=t, op=OP.mult)

            def layer_norm_to(t_in, rows, out_tile):
                """LN over the free dim of t_in[:rows] -> out_tile (casts)."""
                st = stats_pool.tile([128, ND, nc.vector.BN_STATS_DIM], f32, tag="st")
                for j in range(ND):
                    nc.vector.bn_stats(out=st[:rows, j, :],
                                       in_=t_in[:rows, j * DW:(j + 1) * DW])
                mvr = stats_pool.tile([128, 5], f32, tag="mvr")
                nc.vector.bn_aggr(out=mvr[:rows, 0:2], in_=st[:rows])
                ln_rsqrt(mvr, rows)
                nc.vector.tensor_scalar(
                    out=out_tile[:rows], in0=t_in[:rows],
                    scalar1=mvr[:rows, 0:1], scalar2=mvr[:rows, 2:3],
                    op0=OP.subtract, op1=OP.mult)

            ve_nat = persist.tile([F, D], f32)          # LN'd visual embedding
            veT = persist.tile([128, KC, F], f16)       # transposed, for adjacency
            oe_nat = persist.tile([128, NCH, D], f16)   # LN'd object embeddings
            psum_w = persist.tile([F, NW + 3], f32)     # per-window exp sums

            with tc.tile_pool(name="wo", bufs=1) as wop, \
                 tc.tile_pool(name="objs", bufs=3) as objp, \
                 tc.tile_pool(name="ew", bufs=1) as ewp:
                wo = wop.tile([128, KC * D], f16)

                # DMA plan: objT loads ride the scalar HWDGE queue; W_o,
                # then W_v, then all transposes stream on the sync queue.
                obj_tiles = {}

                def load_objT(nch, eng=None):
                    t = objp.tile([128, KC, 128], f16, name="objT", tag="objT")
                    (eng or nc.scalar).dma_start(out=t, in_=objT_d[nch])
                    obj_tiles[nch] = t

                load_objT(0)
                load_objT(1)
                for kc in range(KC):
                    nc.sync.dma_start(out=wo[:, kc * D:(kc + 1) * D],
                                      in_=wo_d[:, kc * D:(kc + 1) * D])

                # ---- chunks 0-1: kc-outer across both chunks ----------
                # 8 PSUM accumulators live so each arriving W_o k-slice
                # feeds 8 matmuls (1.7us PE work per 1.4us DMA): the PE
                # tracks the W_o stream instead of stalling behind it.
                with tc.tile_pool(name="ps01", bufs=1, space="PSUM") as ps01, \
                     tc.tile_pool(name="t01", bufs=2) as t01p:
                    pq01 = {}
                    for c in range(2):
                        for q in range(ND):
                            pq01[c, q] = ps01.tile([128, DW], f32,
                                                   tag=f"q{c}{q}", name=f"pq{c}{q}")
                    for kc in range(KC):
                        for c in range(2):
                            for q in range(ND):
                                nc.tensor.matmul(
                                    pq01[c, q],
                                    lhsT=obj_tiles[c][:, kc, :],
                                    rhs=wo[:, kc * D + q * DW: kc * D + (q + 1) * DW],
                                    start=(kc == 0), stop=(kc == KC - 1))
                    for c in range(2):
                        obj_tiles.pop(c)
                        tB01 = t01p.tile([128, D], f16, tag="tB01", name=f"t01_{c}")
                        for q in range(ND):
                            nc.scalar.activation(out=tB01[:, q * DW:(q + 1) * DW],
                                                 in_=pq01[c, q], func=AF.Tanh)
                        layer_norm_to(tB01, 128, oe_nat[:, c, :])
                    # objT[2] queues on sync BEHIND W_o so it doesn't steal
                    # HBM bandwidth from the W_o stream that paces chunks
                    # 0-1; objT[3]'s buffer WAR delays it naturally.
                    load_objT(2, eng=nc.sync)
                    load_objT(3)

                win_tiles = {}
                en_tiles = {}
                pending_transpose = [0, 1]

                with tc.tile_pool(name="psB", bufs=3, space="PSUM") as psB, \
                     tc.tile_pool(name="psC", bufs=1, space="PSUM") as psC, \
                     tc.tile_pool(name="tmpB", bufs=2) as tmpB:

                    def emit_transpose(nch):
                        # alternate queues so two transposes run in parallel
                        # (DMA_TRANSPOSE occupies the issuing engine ~1.7us)
                        w = nch // 2
                        if w not in win_tiles:
                            win_tiles[w] = tc_win.tile([128, 2, KC, 128], f16,
                                                       name="winT", tag="winT")
                        nc.sync.dma_start(out=win_tiles[w][:, nch % 2, :, :],
                                          in_=oe_nat[:, nch, :], transpose=True)

                    def emit_chunk_B(nch):
                        objT_nc = obj_tiles.pop(nch)
                        if nch + 2 < NCH:
                            load_objT(nch + 2)
                        tB = tmpB.tile([128, D], f16, tag="tB")
                        # quarter-width PSUM tiles (1 bank each, 3 bufs) so each
                        # quarter's tanh overlaps the next quarter's matmuls.
                        for q in range(ND):
                            pq = psB.tile([128, DW], f32, tag="psb")
                            for kc in range(KC):
                                nc.tensor.matmul(
                                    pq,
                                    lhsT=objT_nc[:, kc, :],
                                    rhs=wo[:, kc * D + q * DW: kc * D + (q + 1) * DW],
                                    start=(kc == 0), stop=(kc == KC - 1))
                            nc.scalar.activation(out=tB[:, q * DW:(q + 1) * DW],
                                                 in_=pq, func=AF.Tanh)
                        layer_norm_to(tB, 128, oe_nat[:, nch, :])

                    def emit_window_C(w):
                        """Adjacency + exp for window w (chunks 2w, 2w+1)."""
                        wt = win_tiles.pop(w)
                        padj = psC.tile([F, 256], f32, tag="padj")
                        for kc in range(KC):
                            nc.tensor.matmul(
                                padj,
                                lhsT=veT[:, kc, :],
                                rhs=wt[:, :, kc, :],
                                start=(kc == 0), stop=(kc == KC - 1))
                        # Unnormalized softmax weights: logits are O(1)-bounded
                        # so exp without max-subtraction is safe; accum_out
                        # collects this window's exp-sum for free.  Exp writes
                        # fp16 directly (accumulator stays fp32).
                        e16 = ewp.tile([F, 256], f16, tag="e16")
                        nc.scalar.activation(out=e16, in_=padj, func=AF.Exp,
                                             scale=inv_sqrt_d,
                                             accum_out=psum_w[:, w:w + 1])
                        en = ewp.tile([128, 2, F], f16, tag="en", bufs=2)
                        # [64, 256] -> rows n: [nw, j, f]
                        nc.sync.dma_start(out=en, in_=e16, transpose=True)
                        en_tiles[w] = en

                    def emit_window_D(w):
                        """Aggregation matmuls for window w into ps_agg."""
                        en = en_tiles.pop(w)
                        for j in range(2):
                            for dd in range(ND):
                                nc.tensor.matmul(
                                    ps_agg[:, dd * DW:(dd + 1) * DW],
                                    lhsT=en[:, j, :],
                                    rhs=oe_nat[:, 2 * w + j, dd * DW:(dd + 1) * DW],
                                    start=(w == 0 and j == 0), stop=False)

                    # ---- object chunks 2-3 (steady-state shape) -------
                    with tc.tile_pool(name="wv", bufs=6) as wvp, \
                         tc.tile_pool(name="vt", bufs=1) as vtp, \
                         tc.tile_pool(name="psA", bufs=1, space="PSUM") as psA, \
                         tc.tile_pool(name="tmpA", bufs=1) as tmpA:
                        vt = vtp.tile([128, KC, F], f16)
                        nc.scalar.dma_start(out=vt, in_=vt_d)

                        # W_v streams behind W_o on the sync queue; phase A's
                        # matmuls (emitted below) consume it at chunk-4 time.
                        wv_slices = []
                        for kc in range(KC):
                            wv_k = wvp.tile([128, D], f16, tag="wvk")
                            nc.sync.dma_start(out=wv_k, in_=wv_d[:, kc * D:(kc + 1) * D])
                            wv_slices.append(wv_k)

                        # ---- chunks 2-3 with phase A interleaved ----------
                        # two A k-groups ride behind each B quarter so the
                        # wv stream is consumed as it lands (wvp ring never
                        # gates the PE).
                        ps_ve = psA.tile([F, D], f32)
                        for nch in range(2, 4):
                            objT_nc = obj_tiles.pop(nch)
                            load_objT(nch + 2)
                            tB = tmpB.tile([128, D], f16, tag="tB")
                            for q in range(ND):
                                pq = psB.tile([128, DW], f32, tag="psb")
                                for kc in range(KC):
                                    nc.tensor.matmul(
                                        pq,
                                        lhsT=objT_nc[:, kc, :],
                                        rhs=wo[:, kc * D + q * DW: kc * D + (q + 1) * DW],
                                        start=(kc == 0), stop=(kc == KC - 1))
                                nc.scalar.activation(out=tB[:, q * DW:(q + 1) * DW],
                                                     in_=pq, func=AF.Tanh)
                                for akc in (((nch - 2) * ND + q) * 2,
                                            ((nch - 2) * ND + q) * 2 + 1):
                                    for dd in range(ND):
                                        nc.tensor.matmul(
                                            ps_ve[:, dd * DW:(dd + 1) * DW],
                                            lhsT=vt[:, akc, :],
                                            rhs=wv_slices[akc][:, dd * DW:(dd + 1) * DW],
                                            start=(akc == 0), stop=(akc == KC - 1))
                            layer_norm_to(tB, 128, oe_nat[:, nch, :])
                            pending_transpose.append(nch)

                        # ---- phase A epilogue -----------------------------
                        tA = tmpA.tile([F, D], f32)
                        nc.scalar.activation(out=tA, in_=ps_ve, func=AF.Tanh)
                        layer_norm_to(tA, F, ve_nat)
                        ve_bf = tmpB.tile([F, D], f16, tag="tB")
                        nc.vector.tensor_copy(out=ve_bf, in_=ve_nat)
                        # [64, 2048] -> rows d=(kc*128+kl): [kl, kc, f]
                        nc.sync.dma_start(out=veT, in_=ve_bf, transpose=True)

                    # ---- object chunks 4-17 with fused C/D ----------------
                    with tc.tile_pool(name="win", bufs=3) as tc_win, \
                         tc.tile_pool(name="psD", bufs=1, space="PSUM") as psD:
                        ps_agg = psD.tile([F, D], f32)

                        fin = {}

                        def final_C_half(h):
                            """Adjacency half h of the final window: matmuls
                            + exp + en transpose."""
                            if h == 0:
                                fin["wt"] = win_tiles.pop(NW - 1)
                                fin["padj"] = psC.tile([F, 256], f32, tag="padj",
                                                       name="padj")
                                fin["e16"] = ewp.tile([F, 256], f16, tag="e16",
                                                      name="e16")
                                fin["en"] = ewp.tile([128, 2, F], f16, tag="en",
                                                     bufs=2, name="en")
                            padj, e16, en = fin["padj"], fin["e16"], fin["en"]
                            for kc in range(KC):
                                nc.tensor.matmul(
                                    padj[:, h * 128:(h + 1) * 128],
                                    lhsT=veT[:, kc, :],
                                    rhs=fin["wt"][:, h:h + 1, kc, :],
                                    start=(kc == 0), stop=(kc == KC - 1))
                            nc.scalar.activation(
                                out=e16[:, h * 128:(h + 1) * 128],
                                in_=padj[:, h * 128:(h + 1) * 128],
                                func=AF.Exp, scale=inv_sqrt_d,
                                accum_out=psum_w[:, NW - 1 + h:NW + h])
                            nc.sync.dma_start(out=en[:, h, :],
                                              in_=e16[:, h * 128:(h + 1) * 128],
                                              transpose=True)

                        for nch in range(4, NCH):
                            emit_chunk_B(nch)
                            # drain deferred chunk 0-3 transposes two at a time
                            # behind the current chunk's matmuls
                            for _ in range(min(2, len(pending_transpose))):
                                emit_transpose(pending_transpose.pop(0))
                            emit_transpose(nch)
                            for kind, w in sched.get(nch, []):
                                (emit_window_C if kind == "C" else emit_window_D)(w)

                        # ---- drain: final window in halves ----------------
                        # exp/enT of half a hide behind D(7); half b's behind
                        # D(last, j=0).
                        final_C_half(0)
                        emit_window_D(NW - 2)
                        final_C_half(1)
                        en = fin["en"]
                        # global softmax denominator (cols 0..NW) -> 1/sum
                        nc.vector.reduce_sum(out=psum_w[:, NW + 1:NW + 2],
                                             in_=psum_w[:, :NW + 1], axis=AX.X)
                        nc.vector.reciprocal(out=psum_w[:, NW + 1:NW + 2],
                                             in_=psum_w[:, NW + 1:NW + 2])
                        # D(last, j=0) with en half a
                        for dd in range(ND):
                            nc.tensor.matmul(
                                ps_agg[:, dd * DW:(dd + 1) * DW],
                                lhsT=en[:, 0, :],
                                rhs=oe_nat[:, 2 * (NW - 1), dd * DW:(dd + 1) * DW],
                                start=False, stop=False)

                        # ---- D(last, j=1) per-quarter + pipelined finalize
                        tD = tc_win.tile([F, D], f32, tag="winT")
                        st_f = stats_pool.tile([128, ND, nc.vector.BN_STATS_DIM],
                                               f32, tag="st")
                        for dd in range(ND):
                            nc.tensor.matmul(
                                ps_agg[:, dd * DW:(dd + 1) * DW],
                                lhsT=en[:, 1, :],
                                rhs=oe_nat[:, 2 * NW - 1, dd * DW:(dd + 1) * DW],
                                start=False, stop=True)
                            nc.vector.scalar_tensor_tensor(
                                out=tD[:, dd * DW:(dd + 1) * DW],
                                in0=ps_agg[:, dd * DW:(dd + 1) * DW],
                                scalar=psum_w[:, NW + 1:NW + 2],
                                in1=ve_nat[:, dd * DW:(dd + 1) * DW],
                                op0=OP.mult, op1=OP.add)
                            nc.scalar.activation(out=tD[:, dd * DW:(dd + 1) * DW],
                                                 in_=tD[:, dd * DW:(dd + 1) * DW],
                                                 func=AF.Tanh)
                            nc.vector.bn_stats(out=st_f[:F, dd, :],
                                               in_=tD[:, dd * DW:(dd + 1) * DW])
                        mvr_f = stats_pool.tile([128, 5], f32, tag="mvr")
                        nc.vector.bn_aggr(out=mvr_f[:F, 0:2], in_=st_f[:F])
                        ln_rsqrt(mvr_f, F)
                        # final apply + store in halves: the two output DMAs
                        # go out on different queues so descriptor generation
                        # (~0.6us each) and the transfers run in parallel.
                        out_f = tc_win.tile([F, D], f32, tag="winT")
                        H = D // 2
                        for h, eng in ((0, nc.sync), (1, nc.scalar)):
                            nc.vector.tensor_scalar(
                                out=out_f[:, h * H:(h + 1) * H],
                                in0=tD[:, h * H:(h + 1) * H],
                                scalar1=mvr_f[:F, 0:1], scalar2=mvr_f[:F, 2:3],
                                op0=OP.subtract, op1=OP.mult)
                            eng.dma_start(out=out_d[:, h * H:(h + 1) * H],
                                          in_=out_f[:, h * H:(h + 1) * H])

    nc.compile()
    _BUILD_CACHE["nc"] = nc
    return nc


def _numpy_fallback(inputs):
    """Exact fp32 implementation for non-trivial bias/gain fills."""
    def ln(x, g, b, eps=LN_EPS):
        mu = x.mean(-1, keepdims=True)
        var = x.var(-1, keepdims=True)
        return (x - mu) / np.sqrt(var + eps) * g + b

    vf = _f32(inputs["visual_feats"])
    of = _f32(inputs["obj_feats"])
    W_v, b_v = _f32(inputs["W_v"]), _f32(inputs["b_v"])
    W_o, b_o = _f32(inputs["W_o"]), _f32(inputs["b_o"])
    out = np.zeros((BS, F, D), np.float32)
    for i in range(BS):
        ve = ln(np.tanh(vf[i] @ W_v + b_v), _f32(inputs["ln_v_g"]), _f32(inputs["ln_v_b"]))
        oe = ln(np.tanh(of[i].reshape(N, D) @ W_o + b_o),
                _f32(inputs["ln_o_g"]), _f32(inputs["ln_o_b"]))
        adj = oe @ ve.T / np.sqrt(D)
        adj = np.exp(adj - adj.max(0, keepdims=True))
        adj /= adj.sum(0, keepdims=True)
        out[i] = ln(np.tanh(adj.T @ oe + ve),
                    _f32(inputs["ln_ov_g"]), _f32(inputs["ln_ov_b"]))
    return out


def _prep_core_inputs(visual, obj_flat, shared):
    """Host-side per-sample layout prep. visual [64,2048] f32, obj_flat [2304,2048] f32."""
    m = {
        "objT": np.ascontiguousarray(
            obj_flat.reshape(NCH, 128, KC, 128).transpose(0, 3, 2, 1)
        ).astype(F16).reshape(NCH, 128, KC * 128),
        "vT": np.ascontiguousarray(
            _klc_layout(np.ascontiguousarray(visual.T))).astype(F16),
    }
    m.update(shared)
    return m


def run_kernel(inputs, trace=False):
    """Returns (out [8, 64, 2048] fp32, exec_time_ns or None)."""
    from concourse import bass_utils

    vecs = {k: _f32(inputs[k]) for k in
            ["b_v", "b_o", "ln_v_b", "ln_o_b", "ln_ov_b"]}
    gains = {k: _f32(inputs[k]) for k in ["ln_v_g", "ln_o_g", "ln_ov_g"]}
    trivial = (all(np.all(v == 0) for v in vecs.values())
               and all(np.all(g == 1) for g in gains.values()))
    if not trivial:
        return _numpy_fallback(inputs), None

    visual = _f32(inputs["visual_feats"])            # [8, 64, 2048]
    obj = _f32(inputs["obj_feats"])                  # [8, 64, 36, 2048]
    W_v = _f32(inputs["W_v"])
    W_o = _f32(inputs["W_o"])

    nc = _build()

    shared = {
        "Wo": np.ascontiguousarray(_klc_layout(W_o)).astype(F16),
        "Wv": np.ascontiguousarray(_klc_layout(W_v)).astype(F16),
    }
    in_maps = [
        _prep_core_inputs(visual[c], obj[c].reshape(N, D), shared)
        for c in range(BS)
    ]

    res = bass_utils.run_bass_kernel_spmd(
        nc, in_maps, core_ids=list(range(BS)), trace=trace)
    out = np.stack([res.results[c]["out"] for c in range(BS)], axis=0)
    return out.astype(np.float32), res.exec_time_ns


def kernel(**inputs):
    out, _ = run_kernel(inputs, trace=False)
    return out



# revision 17
# speedup vs baseline: 1.4722x; 1.4722x over previous
"""Trainium2 Bass kernel for nn_EnhancedObj (gnn_message_passing).

Per batch sample (data-parallel over 8 cores, one sample per core):
    ve  = LN(tanh(visual @ W_v + b_v))                  [64, 2048]
    oe  = LN(tanh(obj_flat @ W_o + b_o))                [2304, 2048]
    adj = softmax_n(oe @ ve^T / sqrt(2048))             [2304, 64]
    out = LN(tanh(adj^T @ oe + ve))                     [64, 2048]

The object-branch matmul (92% of the FLOPs) runs in fp8-e4m3 with
perf_mode=DoubleRow: the PE virtualizes to 128x256, doing 2 fp8 MACs
per cell per cycle.  Inputs are pre-scaled on the host (obj x32,
W_o x1024) to sit in e4m3's sweet spot; the descale (1/32768) is
folded into the Tanh activation's scale.  Simulated end-to-end error
of this quantization is ~2e-3 vs the 2e-2 gate.  The visual branch
must stay fp16 (fp8 there costs 4.6e-2 — ve feeds the output
directly), and C/D stay fp16 too (fp8 DMA-transpose is unsupported,
so oe would need an extra full vector pass to duplicate in fp8).

B-matmul schedule: per 128-row object chunk, kc2-OUTER loops (8
double-k slices of 256) over 4 output quarters, so each DoubleRow
LDWEIGHTS (256 cols, no FWL, ~0.2us) is amortized over 4 matmuls.
Each chunk holds 4 PSUM quarter-banks; a 5-buf quarter pool lets the
next chunk's first quarter start while the previous chunk's last
quarter drains through Tanh.  Chunks 0-1 run kc2-outer ACROSS both
chunks (8 banks) to track the W_o stream as it lands.

The visual-branch (A) and aggregation (D) outputs are partition-split
[128, 1024] (d-halves at base partitions 0 and 64) instead of
[64, 2048], halving their PSUM footprint (2 banks each) so phase 2
fits: 5 B-quarters + C (0.5) + D (2) = 7.5 of 8 banks.  The halves
are reassembled with one small cross-partition DMA each.

oe transposes for the adjacency ride the otherwise-idle GPSIMD DMA
queue.  Softmax uses unnormalized exp weights (logits O(1)-bounded),
rescaled by the global 1/sum at the end.  LayerNorm's rsqrt is a
vector-engine bit-hack + Newton step, keeping the scalar engine
exclusively on Tanh/Exp (one activation table, no table swaps).

The device kernel assumes the spec's deterministic fills (zero biases,
unit gains).  If non-trivial bias/gain vectors are ever passed, we
fall back to an exact fp32 numpy implementation.
"""

import numpy as np
import ml_dtypes

F16 = np.float16
F8 = ml_dtypes.float8_e4m3

BS = 8          # batch (== number of cores)
F = 64          # win_len (frames)
OBJ = 36        # objects per frame
D = 2048        # feature dim
N = F * OBJ     # 2304 objects per sample
NCH = N // 128  # 18 object-row chunks
NW = NCH // 2   # 9 two-chunk adjacency windows
KC = D // 128   # 16 contraction chunks (fp16 matmuls)
KC2 = D // 256  # 8 double-k chunks (fp8 DoubleRow)
DW = 512        # B-matmul quarter width (one PSUM bank of fp32)
ND = D // DW    # 4 output-column groups
DH = D // 2     # 1024: partition-split half width for A and D
LN_EPS = 1e-5
RSQRT_MAGIC = 0x5F3759DF

S_X = 32.0      # obj fp8 pre-scale
S_W = 1024.0    # W_o fp8 pre-scale
PERF_SW = False  # DoubleRow (3D AP) vs DoubleRowSwInterleave layout

_BUILD_CACHE = {}


def _f32(x):
    return np.ascontiguousarray(np.asarray(x), dtype=np.float32)


def _q8(x, s):
    return np.clip(np.asarray(x, np.float32) * s, -240.0, 240.0).astype(F8)


def _klc_layout(w):
    """[D, M] -> [128(kl), KC*M] with element (kl, kc, m) = w[kc*128+kl, m]."""
    d, m = w.shape
    assert d == D
    return w.reshape(KC, 128, m).transpose(1, 0, 2).reshape(128, KC * m)


def _sw_interleave(a):
    """[..., 2, 128] DoubleRow pairs -> [..., 256] SwInterleave layout.

    Logical stationary element (i, m) goes to free position 2*(127-m)+i.
    """
    out = np.empty(a.shape[:-2] + (256,), dtype=a.dtype)
    out[..., 0::2] = a[..., 0, ::-1]
    out[..., 1::2] = a[..., 1, ::-1]
    return out


def _build():
    """Build + compile the SPMD Bass program (trivial-fill fast path)."""
    if "nc" in _BUILD_CACHE:
        return _BUILD_CACHE["nc"]

    import concourse.bacc as bacc
    import concourse.tile as tile
    from concourse import mybir

    f32 = mybir.dt.float32
    f16 = mybir.dt.float16
    f8 = mybir.dt.float8e4
    i32 = mybir.dt.int32
    AF = mybir.ActivationFunctionType
    AX = mybir.AxisListType
    OP = mybir.AluOpType
    DRMODE = (mybir.MatmulPerfMode.DoubleRowSwInterleave if PERF_SW
              else mybir.MatmulPerfMode.DoubleRow)

    nc = bacc.Bacc("TRN2", target_bir_lowering=False, debug=False, num_devices=BS)

    objT_shape = ([NCH, 128, KC2, 256] if PERF_SW
                  else [NCH, 128, KC2, 2, 128])
    objT_d = nc.dram_tensor("objT", objT_shape, f8, kind="ExternalInput").ap()
    wo_d = nc.dram_tensor("Wo", [128, KC2, 2, D], f8, kind="ExternalInput").ap()
    wv_d = nc.dram_tensor("Wv", [128, KC * D], f16, kind="ExternalInput").ap()
    vt_d = nc.dram_tensor("vT", [128, KC * F], f16, kind="ExternalInput").ap()
    out_d = nc.dram_tensor("out", [F, D], f32, kind="ExternalOutput").ap()

    inv_sqrt_d = 1.0 / float(np.sqrt(D))
    descale = 1.0 / (S_X * S_W)

    # adjacency (C) / aggregation (D) emission points: window w covers
    # object chunks (2w, 2w+1); C(w) needs both transposed + veT (ready
    # ~chunk 4); D(w) follows C(w) one chunk later.  Window NW-1 drains
    # manually after the loop.
    sched = {}
    for w in range(NW):
        c_at = 7 + w if w < 3 else max(2 * w + 3, 10)
        c_at = min(c_at, NCH - 1) if w < NW - 1 else NCH
        d_at = c_at + 1
        if c_at < NCH:
            sched.setdefault(c_at, []).append(("C", w))
        if d_at < NCH:
            sched.setdefault(d_at, []).append(("D", w))

    def lhsT_B(objT_tile, kc2):
        if PERF_SW:
            return objT_tile[:, kc2, :]
        return objT_tile[:, kc2, :, :]

    with tile.TileContext(nc) as tc:
        with tc.tile_pool(name="persist", bufs=1) as persist, \
             tc.tile_pool(name="stats", bufs=2) as stats_pool:

            def ln_rsqrt(mvr, rows):
                """mvr[:,1]=var -> mvr[:,2]=1/sqrt(var+eps), vector engine
                only (bit-hack seed + 1 Newton step; no act-table load)."""
                x, y, t = mvr[:rows, 3:4], mvr[:rows, 2:3], mvr[:rows, 4:5]
                nc.vector.tensor_scalar(out=x, in0=mvr[:rows, 1:2],
                                        scalar1=LN_EPS, scalar2=None, op0=OP.add)
                nc.vector.tensor_scalar(out=t.bitcast(i32), in0=x.bitcast(i32),
                                        scalar1=1, scalar2=None,
                                        op0=OP.logical_shift_right)
                # magic - (i>>1)  ==  (i>>1) * -1 + magic  (both ops arith)
                nc.vector.tensor_scalar(out=y.bitcast(i32), in0=t.bitcast(i32),
                                        scalar1=-1, scalar2=RSQRT_MAGIC,
                                        op0=OP.mult, op1=OP.add)
                nc.vector.tensor_tensor(out=t, in0=y, in1=y, op=OP.mult)
                nc.vector.tensor_tensor(out=t, in0=t, in1=x, op=OP.mult)
                nc.vector.tensor_scalar(out=t, in0=t, scalar1=-0.5,
                                        scalar2=1.5, op0=OP.mult, op1=OP.add)
                nc.vector.tensor_tensor(out=y, in0=y, in1=t, op=OP.mult)

            def layer_norm_to(t_in, rows, out_tile):
                """LN over the free dim of t_in[:rows] -> out_tile (casts)."""
                st = stats_pool.tile([128, ND, nc.vector.BN_STATS_DIM], f32, tag="st")
                for j in range(ND):
                    nc.vector.bn_stats(out=st[:rows, j, :],
                                       in_=t_in[:rows, j * DW:(j + 1) * DW])
                mvr = stats_pool.tile([128, 5], f32, tag="mvr")
                nc.vector.bn_aggr(out=mvr[:rows, 0:2], in_=st[:rows])
                ln_rsqrt(mvr, rows)
                nc.vector.tensor_scalar(
                    out=out_tile[:rows], in0=t_in[:rows],
                    scalar1=mvr[:rows, 0:1], scalar2=mvr[:rows, 2:3],
                    op0=OP.subtract, op1=OP.mult)

            ve_nat = persist.tile([F, D], f32)          # LN'd visual embedding
            veT = persist.tile([128, KC, F], f16)       # transposed, for adjacency
            oe_nat = persist.tile([128, NCH, D], f16)   # LN'd object embeddings
            psum_w = persist.tile([F, NW + 3], f32)     # per-window exp sums

            with tc.tile_pool(name="wo", bufs=1) as wop, \
                 tc.tile_pool(name="objs", bufs=4) as objp, \
                 tc.tile_pool(name="win", bufs=3) as tc_win, \
                 tc.tile_pool(name="ew", bufs=1) as ewp:
                wo = wop.tile([128, KC2, 2, D], f8)

                obj_tiles = {}
                win_tiles = {}
                en_tiles = {}

                # objT (and later wv) ride the otherwise-idle GPSIMD DMA
                # queue; the sync queue carries W_o + the oe transposes.
                def load_objT(nch):
                    t = objp.tile(objT_shape[1:], f8, name="objT", tag="objT")
                    nc.gpsimd.dma_start(out=t, in_=objT_d[nch])
                    obj_tiles[nch] = t

                def win_tiles_tile(nch):
                    w = nch // 2
                    if w not in win_tiles:
                        win_tiles[w] = tc_win.tile([128, 2, KC, 128], f16,
                                                   name="winT", tag="winT")
                    return win_tiles[w][:, nch % 2, :, :]

                def emit_transpose(nch):
                    nc.sync.dma_start(out=win_tiles_tile(nch),
                                      in_=oe_nat[:, nch, :], transpose=True)

                for c in range(4):
                    load_objT(c)
                for kc2 in range(KC2):
                    nc.sync.dma_start(out=wo[:, kc2], in_=wo_d[:, kc2])

                # ---- chunks 0-1: kc2-outer across both chunks ----------
                # 8 PSUM quarter-banks live so each arriving W_o k-slice
                # feeds 8 DoubleRow matmuls: the PE tracks the W_o stream.
                with tc.tile_pool(name="ps01", bufs=1, space="PSUM") as ps01, \
                     tc.tile_pool(name="t01", bufs=2) as t01p:
                    pq01 = {}
                    for c in range(2):
                        for q in range(ND):
                            pq01[c, q] = ps01.tile([128, DW], f32,
                                                   tag=f"q{c}{q}", name=f"pq{c}{q}")
                    for kc2 in range(KC2):
                        for c in range(2):
                            lt = lhsT_B(obj_tiles[c], kc2)
                            for q in range(ND):
                                nc.tensor.matmul(
                                    pq01[c, q],
                                    lhsT=lt,
                                    rhs=wo[:, kc2, :, q * DW:(q + 1) * DW],
                                    start=(kc2 == 0), stop=(kc2 == KC2 - 1),
                                    perf_mode=DRMODE)
                    for c in range(2):
                        obj_tiles.pop(c)
                        tB01 = t01p.tile([128, D], f16, tag="tB01", name=f"t01_{c}")
                        for q in range(ND):
                            nc.scalar.activation(out=tB01[:, q * DW:(q + 1) * DW],
                                                 in_=pq01[c, q], func=AF.Tanh,
                                                 scale=descale)
                        layer_norm_to(tB01, 128, oe_nat[:, c, :])
                        emit_transpose(c)

                with tc.tile_pool(name="psB", bufs=5, space="PSUM") as psB, \
                     tc.tile_pool(name="psC", bufs=1, space="PSUM") as psC, \
                     tc.tile_pool(name="tmpB", bufs=2) as tmpB:

                    def emit_chunk_B(nch, a_hook=None):
                        objT_nc = obj_tiles.pop(nch)
                        if nch + 2 < NCH:
                            load_objT(nch + 2)
                        tB = tmpB.tile([128, D], f16, tag="tB")
                        pq = [psB.tile([128, DW], f32, tag="psb", name=f"psb{i}")
                              for i in range(ND)]
                        for kc2 in range(KC2):
                            lt = lhsT_B(objT_nc, kc2)
                            for q in range(ND):
                                nc.tensor.matmul(
                                    pq[q],
                                    lhsT=lt,
                                    rhs=wo[:, kc2, :, q * DW:(q + 1) * DW],
                                    start=(kc2 == 0), stop=(kc2 == KC2 - 1),
                                    perf_mode=DRMODE)
                            if a_hook is not None:
                                a_hook(kc2)
                        for q in range(ND):
                            nc.scalar.activation(out=tB[:, q * DW:(q + 1) * DW],
                                                 in_=pq[q], func=AF.Tanh,
                                                 scale=descale)
                        layer_norm_to(tB, 128, oe_nat[:, nch, :])
                        emit_transpose(nch)

                    def emit_window_C(w):
                        """Adjacency + exp for window w (chunks 2w, 2w+1)."""
                        wt = win_tiles.pop(w)
                        padj = psC.tile([F, 256], f32, tag="padj")
                        for kc in range(KC):
                            nc.tensor.matmul(
                                padj,
                                lhsT=veT[:, kc, :],
                                rhs=wt[:, :, kc, :],
                                start=(kc == 0), stop=(kc == KC - 1))
                        # Unnormalized softmax weights: logits are O(1)-bounded
                        # so exp without max-subtraction is safe; accum_out
                        # collects this window's exp-sum for free.
                        e16 = ewp.tile([F, 256], f16, tag="e16")
                        nc.scalar.activation(out=e16, in_=padj, func=AF.Exp,
                                             scale=inv_sqrt_d,
                                             accum_out=psum_w[:, w:w + 1])
                        en = ewp.tile([128, 2, F], f16, tag="en", bufs=2)
                        # [64, 256] -> rows n: [nw, j, f]
                        nc.scalar.dma_start(out=en, in_=e16, transpose=True)
                        en_tiles[w] = en

                    def emit_window_D(w):
                        """Aggregation matmuls for window w into split ps_agg."""
                        en = en_tiles.pop(w)
                        for j in range(2):
                            for h in range(2):
                                for u in range(2):
                                    nc.tensor.matmul(
                                        ps_agg[h * F:(h + 1) * F,
                                               u * DW:(u + 1) * DW],
                                        lhsT=en[:, j, :],
                                        rhs=oe_nat[:, 2 * w + j,
                                                   h * DH + u * DW:
                                                   h * DH + (u + 1) * DW],
                                        start=(w == 0 and j == 0), stop=False)

                    # ---- chunks 2-5 with visual branch (A) interleaved ----
                    # A output is partition-split [128, 1024] (d-halves at
                    # base partitions 0/64) so psA is 2 banks, fitting next
                    # to the 5-buf B quarter pool.  One A k-slice rides every
                    # other kc2 slot, pacing wv's fp16 HBM stream (8.4MB)
                    # across four chunks instead of two.
                    with tc.tile_pool(name="wv", bufs=6) as wvp, \
                         tc.tile_pool(name="vt", bufs=1) as vtp, \
                         tc.tile_pool(name="psA", bufs=1, space="PSUM") as psA, \
                         tc.tile_pool(name="tmpA", bufs=1) as tmpA:
                        vt = vtp.tile([128, KC, F], f16)
                        nc.scalar.dma_start(out=vt, in_=vt_d)

                        wv_slices = []

                        def load_wv():
                            kc = len(wv_slices)
                            wv_k = wvp.tile([128, D], f16, tag="wvk")
                            nc.gpsimd.dma_start(out=wv_k,
                                                in_=wv_d[:, kc * D:(kc + 1) * D])
                            wv_slices.append(wv_k)

                        for _ in range(3):
                            load_wv()

                        ps_ve = psA.tile([128, DH], f32)

                        def a_hook_for(nch):
                            def hook(kc2):
                                if kc2 % 2:
                                    return
                                akc = (nch - 2) * (KC2 // 2) + kc2 // 2
                                for h in range(2):
                                    for u in range(2):
                                        nc.tensor.matmul(
                                            ps_ve[h * F:(h + 1) * F,
                                                  u * DW:(u + 1) * DW],
                                            lhsT=vt[:, akc, :],
                                            rhs=wv_slices[akc][
                                                :, h * DH + u * DW:
                                                h * DH + (u + 1) * DW],
                                            start=(akc == 0),
                                            stop=(akc == KC - 1))
                                if len(wv_slices) < KC:
                                    load_wv()
                            return hook

                        for nch in range(2, 6):
                            emit_chunk_B(nch, a_hook=a_hook_for(nch))

                        # ---- phase A epilogue -----------------------------
                        # tanh the split halves; reassemble with one
                        # cross-partition DMA; LN in natural layout.
                        tA = tmpA.tile([F, D], f32)
                        tA2 = tmpA.tile([128, DH], f32)
                        nc.scalar.activation(out=tA2[F:, :], in_=ps_ve[F:, :],
                                             func=AF.Tanh)
                        nc.scalar.dma_start(out=tA[:, DH:], in_=tA2[F:, :])
                        nc.scalar.activation(out=tA[:, :DH], in_=ps_ve[:F, :],
                                             func=AF.Tanh)
                        layer_norm_to(tA, F, ve_nat)
                        ve_bf = tmpB.tile([F, D], f16, tag="tB")
                        nc.vector.tensor_copy(out=ve_bf, in_=ve_nat)
                        # [64, 2048] -> rows d=(kc*128+kl): [kl, kc, f]
                        nc.sync.dma_start(out=veT, in_=ve_bf, transpose=True)

                    # ---- object chunks 6-17 with fused C/D ----------------
                    with tc.tile_pool(name="psD", bufs=1, space="PSUM") as psD:
                        ps_agg = psD.tile([128, DH], f32)

                        fin = {}

                        def final_C_half(h):
                            """Adjacency half h of the final window: matmuls
                            + exp + en transpose."""
                            if h == 0:
                                fin["wt"] = win_tiles.pop(NW - 1)
                                fin["padj"] = psC.tile([F, 256], f32, tag="padj",
                                                       name="padj")
                                fin["e16"] = ewp.tile([F, 256], f16, tag="e16",
                                                      name="e16")
                                fin["en"] = ewp.tile([128, 2, F], f16, tag="en",
                                                     bufs=2, name="en")
                            padj, e16, en = fin["padj"], fin["e16"], fin["en"]
                            for kc in range(KC):
                                nc.tensor.matmul(
                                    padj[:, h * 128:(h + 1) * 128],
                                    lhsT=veT[:, kc, :],
                                    rhs=fin["wt"][:, h:h + 1, kc, :],
                                    start=(kc == 0), stop=(kc == KC - 1))
                            nc.scalar.activation(
                                out=e16[:, h * 128:(h + 1) * 128],
                                in_=padj[:, h * 128:(h + 1) * 128],
                                func=AF.Exp, scale=inv_sqrt_d,
                                accum_out=psum_w[:, NW - 1 + h:NW + h])
                            nc.sync.dma_start(out=en[:, h, :],
                                              in_=e16[:, h * 128:(h + 1) * 128],
                                              transpose=True)

                        for nch in range(6, NCH):
                            emit_chunk_B(nch)
                            for kind, w in sched.get(nch, []):
                                (emit_window_C if kind == "C" else emit_window_D)(w)

                        # ---- drain: final window in halves ----------------
                        final_C_half(0)
                        emit_window_D(NW - 2)
                        final_C_half(1)
                        en = fin["en"]
                        # global softmax denominator (cols 0..NW) -> 1/sum
                        nc.vector.reduce_sum(out=psum_w[:, NW + 1:NW + 2],
                                             in_=psum_w[:, :NW + 1], axis=AX.X)
                        nc.vector.reciprocal(out=psum_w[:, NW + 1:NW + 2],
                                             in_=psum_w[:, NW + 1:NW + 2])
                        # final D window: h=1 half first so its raw merge DMA
                        # overlaps the h=0 matmuls and rescale.
                        tD = tc_win.tile([F, D], f32, tag="winT")
                        for j in range(2):
                            for u in range(2):
                                nc.tensor.matmul(
                                    ps_agg[F:, u * DW:(u + 1) * DW],
                                    lhsT=en[:, j, :],
                                    rhs=oe_nat[:, 2 * NW - 2 + j,
                                               DH + u * DW:DH + (u + 1) * DW],
                                    start=False, stop=(j == 1))
                        # DMA cannot read PSUM: bounce h1 through SBUF
                        drn = tmpB.tile([128, DH], f32, tag="drn", bufs=1)
                        nc.vector.tensor_copy(out=drn[F:, :], in_=ps_agg[F:, :])
                        nc.scalar.dma_start(out=tD[:, DH:], in_=drn[F:, :])
                        for j in range(2):
                            for u in range(2):
                                nc.tensor.matmul(
                                    ps_agg[:F, u * DW:(u + 1) * DW],
                                    lhsT=en[:, j, :],
                                    rhs=oe_nat[:, 2 * NW - 2 + j,
                                               u * DW:(u + 1) * DW],
                                    start=False, stop=(j == 1))
                        # rescale by global 1/sum + add ve (both halves now on
                        # partitions 0-63), tanh, final LN
                        nc.vector.scalar_tensor_tensor(
                            out=tD[:, :DH], in0=ps_agg[:F, :],
                            scalar=psum_w[:, NW + 1:NW + 2],
                            in1=ve_nat[:, :DH], op0=OP.mult, op1=OP.add)
                        nc.vector.scalar_tensor_tensor(
                            out=tD[:, DH:], in0=tD[:, DH:],
                            scalar=psum_w[:, NW + 1:NW + 2],
                            in1=ve_nat[:, DH:], op0=OP.mult, op1=OP.add)
                        nc.scalar.activation(out=tD, in_=tD, func=AF.Tanh)
                        st_f = stats_pool.tile([128, ND, nc.vector.BN_STATS_DIM],
                                               f32, tag="st")
                        for dd in range(ND):
                            nc.vector.bn_stats(out=st_f[:F, dd, :],
                                               in_=tD[:, dd * DW:(dd + 1) * DW])
                        mvr_f = stats_pool.tile([128, 5], f32, tag="mvr")
                        nc.vector.bn_aggr(out=mvr_f[:F, 0:2], in_=st_f[:F])
                        ln_rsqrt(mvr_f, F)
                        # final apply + store in halves on separate queues
                        out_f = tc_win.tile([F, D], f32, tag="winT")
                        for h, eng in ((0, nc.sync), (1, nc.scalar)):
                            nc.vector.tensor_scalar(
                                out=out_f[:, h * DH:(h + 1) * DH],
                                in0=tD[:, h * DH:(h + 1) * DH],
                                scalar1=mvr_f[:F, 0:1], scalar2=mvr_f[:F, 2:3],
                                op0=OP.subtract, op1=OP.mult)
                            eng.dma_start(out=out_d[:, h * DH:(h + 1) * DH],
                                          in_=out_f[:, h * DH:(h + 1) * DH])

    nc.compile()
    _BUILD_CACHE["nc"] = nc
    return nc


def _numpy_fallback(inputs):
    """Exact fp32 implementation for non-trivial bias/gain fills."""
    def ln(x, g, b, eps=LN_EPS):
        mu = x.mean(-1, keepdims=True)
        var = x.var(-1, keepdims=True)
        return (x - mu) / np.sqrt(var + eps) * g + b

    vf = _f32(inputs["visual_feats"])
    of = _f32(inputs["obj_feats"])
    W_v, b_v = _f32(inputs["W_v"]), _f32(inputs["b_v"])
    W_o, b_o = _f32(inputs["W_o"]), _f32(inputs["b_o"])
    out = np.zeros((BS, F, D), np.float32)
    for i in range(BS):
        ve = ln(np.tanh(vf[i] @ W_v + b_v), _f32(inputs["ln_v_g"]), _f32(inputs["ln_v_b"]))
        oe = ln(np.tanh(of[i].reshape(N, D) @ W_o + b_o),
                _f32(inputs["ln_o_g"]), _f32(inputs["ln_o_b"]))
        adj = oe @ ve.T / np.sqrt(D)
        adj = np.exp(adj - adj.max(0, keepdims=True))
        adj /= adj.sum(0, keepdims=True)
        out[i] = ln(np.tanh(adj.T @ oe + ve),
                    _f32(inputs["ln_ov_g"]), _f32(inputs["ln_ov_b"]))
    return out


def _prep_core_inputs(visual, obj_flat, shared):
    """Per-sample layout prep. visual [64,2048] f32, obj_flat [2304,2048] f32.

    objT DoubleRow layout: [NCH, 128(kl), KC2, 2, 128(n)] with element
    (nch, kl, kc2, i, n) = obj[nch*128+n, kc2*256+i*128+kl] * S_X.
    """
    ot = _q8(obj_flat, S_X).reshape(NCH, 128, KC2, 2, 128).transpose(0, 4, 2, 3, 1)
    if PERF_SW:
        ot = _sw_interleave(np.ascontiguousarray(ot))
    m = {
        "objT": np.ascontiguousarray(ot),
        "vT": np.ascontiguousarray(
            _klc_layout(np.ascontiguousarray(visual.T))).astype(F16),
    }
    m.update(shared)
    return m


def run_kernel(inputs, trace=False):
    """Returns (out [8, 64, 2048] fp32, exec_time_ns or None)."""
    from concourse import bass_utils

    vecs = {k: _f32(inputs[k]) for k in
            ["b_v", "b_o", "ln_v_b", "ln_o_b", "ln_ov_b"]}
    gains = {k: _f32(inputs[k]) for k in ["ln_v_g", "ln_o_g", "ln_ov_g"]}
    trivial = (all(np.all(v == 0) for v in vecs.values())
               and all(np.all(g == 1) for g in gains.values()))
    if not trivial:
        return _numpy_fallback(inputs), None

    visual = _f32(inputs["visual_feats"])            # [8, 64, 2048]
    obj = _f32(inputs["obj_feats"])                  # [8, 64, 36, 2048]
    W_v = _f32(inputs["W_v"])
    W_o = _f32(inputs["W_o"])

    nc = _build()

    # Wo fp8 layout: [128(kl), KC2, 2, D] with (kl, kc2, i, m) =
    # W_o[kc2*256+i*128+kl, m] * S_W.
    wo8 = np.ascontiguousarray(
        _q8(W_o, S_W).reshape(KC2, 2, 128, D).transpose(2, 0, 1, 3))
    shared = {
        "Wo": wo8,
        "Wv": np.ascontiguousarray(_klc_layout(W_v)).astype(F16),
    }
    in_maps = [
        _prep_core_inputs(visual[c], obj[c].reshape(N, D), shared)
        for c in range(BS)
    ]

    res = bass_utils.run_bass_kernel_spmd(
        nc, in_maps, core_ids=list(range(BS)), trace=trace)
    out = np.stack([res.results[c]["out"] for c in range(BS)], axis=0)
    return out.astype(np.float32), res.exec_time_ns


def kernel(**inputs):
    out, _ = run_kernel(inputs, trace=False)
    return out


# revision 21
# speedup vs baseline: 1.4783x; 1.0042x over previous
"""Trainium2 Bass kernel for nn_EnhancedObj (gnn_message_passing).

Per batch sample (data-parallel over 8 cores, one sample per core):
    ve  = LN(tanh(visual @ W_v + b_v))                  [64, 2048]
    oe  = LN(tanh(obj_flat @ W_o + b_o))                [2304, 2048]
    adj = softmax_n(oe @ ve^T / sqrt(2048))             [2304, 64]
    out = LN(tanh(adj^T @ oe + ve))                     [64, 2048]

The object-branch matmul (92% of the FLOPs) runs in fp8-e4m3 with
perf_mode=DoubleRow: the PE virtualizes to 128x256, doing 2 fp8 MACs
per cell per cycle.  Inputs are pre-scaled on the host (obj x32,
W_o x1024) to sit in e4m3's sweet spot; the descale (1/32768) is
folded into the Tanh activation's scale.  Simulated end-to-end error
of this quantization is ~2e-3 vs the 2e-2 gate.  The visual branch
must stay fp16 (fp8 there costs 4.6e-2 — ve feeds the output
directly), and C/D stay fp16 too (fp8 DMA-transpose is unsupported,
so oe would need an extra full vector pass to duplicate in fp8).

B-matmul schedule: per 128-row object chunk, kc2-OUTER loops (8
double-k slices of 256) over 4 output quarters, so each DoubleRow
LDWEIGHTS (256 cols, no FWL, ~0.2us) is amortized over 4 matmuls.
Each chunk holds 4 PSUM quarter-banks; a 5-buf quarter pool lets the
next chunk's first quarter start while the previous chunk's last
quarter drains through Tanh.  Chunks 0-1 run kc2-outer ACROSS both
chunks (8 banks) to track the W_o stream as it lands.

The visual-branch (A) and aggregation (D) outputs are partition-split
[128, 1024] (d-halves at base partitions 0 and 64) instead of
[64, 2048], halving their PSUM footprint (2 banks each) so phase 2
fits: 5 B-quarters + C (0.5) + D (2) = 7.5 of 8 banks.  The halves
are reassembled with one small cross-partition DMA each.

oe transposes for the adjacency ride the otherwise-idle GPSIMD DMA
queue.  Softmax uses unnormalized exp weights (logits O(1)-bounded),
rescaled by the global 1/sum at the end.  LayerNorm's rsqrt is a
vector-engine bit-hack + Newton step, keeping the scalar engine
exclusively on Tanh/Exp (one activation table, no table swaps).

The device kernel assumes the spec's deterministic fills (zero biases,
unit gains).  If non-trivial bias/gain vectors are ever passed, we
fall back to an exact fp32 numpy implementation.
"""

import numpy as np
import ml_dtypes

F16 = np.float16
F8 = ml_dtypes.float8_e4m3

BS = 8          # batch (== number of cores)
F = 64          # win_len (frames)
OBJ = 36        # objects per frame
D = 2048        # feature dim
N = F * OBJ     # 2304 objects per sample
NCH = N // 128  # 18 object-row chunks
NW = NCH // 2   # 9 two-chunk adjacency windows
KC = D // 128   # 16 contraction chunks (fp16 matmuls)
KC2 = D // 256  # 8 double-k chunks (fp8 DoubleRow)
DW = 512        # B-matmul quarter width (one PSUM bank of fp32)
ND = D // DW    # 4 output-column groups
DH = D // 2     # 1024: partition-split half width for A and D
LN_EPS = 1e-5
RSQRT_MAGIC = 0x5F3759DF

S_X = 32.0      # obj fp8 pre-scale
S_W = 1024.0    # W_o fp8 pre-scale
PERF_SW = False  # DoubleRow (3D AP) vs DoubleRowSwInterleave layout
DEBUG = False    # add per-stage debug outputs

_BUILD_CACHE = {}


def _f32(x):
    return np.ascontiguousarray(np.asarray(x), dtype=np.float32)


def _q8(x, s):
    return np.clip(np.asarray(x, np.float32) * s, -240.0, 240.0).astype(F8)


def _klc_layout(w):
    """[D, M] -> [128(kl), KC*M] with element (kl, kc, m) = w[kc*128+kl, m]."""
    d, m = w.shape
    assert d == D
    return w.reshape(KC, 128, m).transpose(1, 0, 2).reshape(128, KC * m)


def _sw_interleave(a):
    """[..., 2, 128] DoubleRow pairs -> [..., 256] SwInterleave layout.

    Logical stationary element (i, m) goes to free position 2*(127-m)+i.
    """
    out = np.empty(a.shape[:-2] + (256,), dtype=a.dtype)
    out[..., 0::2] = a[..., 0, ::-1]
    out[..., 1::2] = a[..., 1, ::-1]
    return out


def _build():
    """Build + compile the SPMD Bass program (trivial-fill fast path)."""
    if "nc" in _BUILD_CACHE:
        return _BUILD_CACHE["nc"]

    import concourse.bacc as bacc
    import concourse.tile as tile
    from concourse import mybir

    f32 = mybir.dt.float32
    f16 = mybir.dt.float16
    f8 = mybir.dt.float8e4
    i32 = mybir.dt.int32
    AF = mybir.ActivationFunctionType
    AX = mybir.AxisListType
    OP = mybir.AluOpType
    DRMODE = (mybir.MatmulPerfMode.DoubleRowSwInterleave if PERF_SW
              else mybir.MatmulPerfMode.DoubleRow)

    nc = bacc.Bacc("TRN2", target_bir_lowering=False, debug=False, num_devices=BS)

    objT_shape = ([NCH, 128, KC2, 256] if PERF_SW
                  else [NCH, 128, KC2, 2, 128])
    objT_d = nc.dram_tensor("objT", objT_shape, f8, kind="ExternalInput").ap()
    wo_d = nc.dram_tensor("Wo", [128, KC2, 2, D], f8, kind="ExternalInput").ap()
    wv_d = nc.dram_tensor("Wv", [128, KC * D], f16, kind="ExternalInput").ap()
    vt_d = nc.dram_tensor("vT", [128, KC * F], f16, kind="ExternalInput").ap()
    out_d = nc.dram_tensor("out", [F, D], f32, kind="ExternalOutput").ap()
    if DEBUG:
        dve_d = nc.dram_tensor("dve", [F, D], f32, kind="ExternalOutput").ap()
        de16_d = nc.dram_tensor("de16", [NW, F, 256], f16, kind="ExternalOutput").ap()
        dpw_d = nc.dram_tensor("dpw", [F, NW + 3], f32, kind="ExternalOutput").ap()
        doe_d = nc.dram_tensor("doe", [128, NCH, D], f16, kind="ExternalOutput").ap()

    inv_sqrt_d = 1.0 / float(np.sqrt(D))
    descale = 1.0 / (S_X * S_W)

    # adjacency (C) / aggregation (D) emission points: window w covers
    # object chunks (2w, 2w+1); C(w) needs both transposed + veT (ready
    # ~chunk 4); D(w) follows C(w) one chunk later.  Window NW-1 drains
    # manually after the loop.
    sched = {}
    for w in range(NW):
        c_at = 7 + w if w < 3 else max(2 * w + 3, 10)
        c_at = min(c_at, NCH - 1) if w < NW - 1 else NCH
        d_at = c_at + 1
        if c_at < NCH:
            sched.setdefault(c_at, []).append(("C", w))
        if d_at < NCH:
            sched.setdefault(d_at, []).append(("D", w))

    def lhsT_B(objT_tile, kc2):
        if PERF_SW:
            return objT_tile[:, kc2, :]
        return objT_tile[:, kc2, :, :]

    with tile.TileContext(nc) as tc:
        with tc.tile_pool(name="persist", bufs=1) as persist, \
             tc.tile_pool(name="stats", bufs=2) as stats_pool:

            def ln_rsqrt(mvr, rows):
                """mvr[:,1]=var -> mvr[:,2]=1/sqrt(var+eps), vector engine
                only (bit-hack seed + 1 Newton step; no act-table load)."""
                x, y, t = mvr[:rows, 3:4], mvr[:rows, 2:3], mvr[:rows, 4:5]
                nc.vector.tensor_scalar(out=x, in0=mvr[:rows, 1:2],
                                        scalar1=LN_EPS, scalar2=None, op0=OP.add)
                nc.vector.tensor_scalar(out=t.bitcast(i32), in0=x.bitcast(i32),
                                        scalar1=1, scalar2=None,
                                        op0=OP.logical_shift_right)
                # magic - (i>>1)  ==  (i>>1) * -1 + magic  (both ops arith)
                nc.vector.tensor_scalar(out=y.bitcast(i32), in0=t.bitcast(i32),
                                        scalar1=-1, scalar2=RSQRT_MAGIC,
                                        op0=OP.mult, op1=OP.add)
                nc.vector.tensor_tensor(out=t, in0=y, in1=y, op=OP.mult)
                nc.vector.tensor_tensor(out=t, in0=t, in1=x, op=OP.mult)
                nc.vector.tensor_scalar(out=t, in0=t, scalar1=-0.5,
                                        scalar2=1.5, op0=OP.mult, op1=OP.add)
                nc.vector.tensor_tensor(out=y, in0=y, in1=t, op=OP.mult)

            def layer_norm_to(t_in, rows, out_tile):
                """LN over the free dim of t_in[:rows] -> out_tile (casts)."""
                st = stats_pool.tile([128, ND, nc.vector.BN_STATS_DIM], f32, tag="st")
                for j in range(ND):
                    nc.vector.bn_stats(out=st[:rows, j, :],
                                       in_=t_in[:rows, j * DW:(j + 1) * DW])
                mvr = stats_pool.tile([128, 5], f32, tag="mvr")
                nc.vector.bn_aggr(out=mvr[:rows, 0:2], in_=st[:rows])
                ln_rsqrt(mvr, rows)
                nc.vector.tensor_scalar(
                    out=out_tile[:rows], in0=t_in[:rows],
                    scalar1=mvr[:rows, 0:1], scalar2=mvr[:rows, 2:3],
                    op0=OP.subtract, op1=OP.mult)

            ve_nat = persist.tile([F, D], f32)          # LN'd visual embedding
            veT = persist.tile([128, KC, F], f16)       # transposed, for adjacency
            oe_nat = persist.tile([128, NCH, D], f16)   # LN'd object embeddings
            psum_w = persist.tile([F, NW + 3], f32)     # per-window exp sums

            with tc.tile_pool(name="wo", bufs=1) as wop, \
                 tc.tile_pool(name="objs", bufs=4) as objp, \
                 tc.tile_pool(name="win", bufs=4) as tc_win, \
                 tc.tile_pool(name="ew", bufs=1) as ewp:
                wo = wop.tile([128, KC2, 2, D], f8)

                obj_tiles = {}
                win_tiles = {}
                en_tiles = {}

                # objT (and later wv) ride the otherwise-idle GPSIMD DMA
                # queue; the sync queue carries W_o + the oe transposes.
                def load_objT(nch):
                    t = objp.tile(objT_shape[1:], f8, name="objT", tag="objT")
                    nc.gpsimd.dma_start(out=t, in_=objT_d[nch])
                    obj_tiles[nch] = t

                def win_tiles_tile(nch):
                    w = nch // 2
                    if w not in win_tiles:
                        win_tiles[w] = tc_win.tile([128, 2, KC, 128], f16,
                                                   name="winT", tag="winT")
                    return win_tiles[w][:, nch % 2, :, :]

                def emit_transpose(nch):
                    nc.sync.dma_start(out=win_tiles_tile(nch),
                                      in_=oe_nat[:, nch, :], transpose=True)

                for c in range(4):
                    load_objT(c)
                for kc2 in range(KC2):
                    nc.sync.dma_start(out=wo[:, kc2], in_=wo_d[:, kc2])

                # ---- chunks 0-1: kc2-outer across both chunks ----------
                # 8 PSUM quarter-banks live so each arriving W_o k-slice
                # feeds 8 DoubleRow matmuls: the PE tracks the W_o stream.
                with tc.tile_pool(name="ps01", bufs=1, space="PSUM") as ps01, \
                     tc.tile_pool(name="t01", bufs=2) as t01p:
                    pq01 = {}
                    for c in range(2):
                        for q in range(ND):
                            pq01[c, q] = ps01.tile([128, DW], f32,
                                                   tag=f"q{c}{q}", name=f"pq{c}{q}")
                    for kc2 in range(KC2):
                        for c in range(2):
                            lt = lhsT_B(obj_tiles[c], kc2)
                            for q in range(ND):
                                nc.tensor.matmul(
                                    pq01[c, q],
                                    lhsT=lt,
                                    rhs=wo[:, kc2, :, q * DW:(q + 1) * DW],
                                    start=(kc2 == 0), stop=(kc2 == KC2 - 1),
                                    perf_mode=DRMODE)
                    for c in range(2):
                        obj_tiles.pop(c)
                        tB01 = t01p.tile([128, D], f16, tag="tB01", name=f"t01_{c}")
                        for q in range(ND):
                            nc.scalar.activation(out=tB01[:, q * DW:(q + 1) * DW],
                                                 in_=pq01[c, q], func=AF.Tanh,
                                                 scale=descale)
                        layer_norm_to(tB01, 128, oe_nat[:, c, :])
                        emit_transpose(c)

                with tc.tile_pool(name="psB", bufs=5, space="PSUM") as psB, \
                     tc.tile_pool(name="psC", bufs=1, space="PSUM") as psC, \
                     tc.tile_pool(name="tmpB", bufs=2) as tmpB:

                    def emit_chunk_B(nch, a_hook=None):
                        objT_nc = obj_tiles.pop(nch)
                        if nch + 2 < NCH:
                            load_objT(nch + 2)
                        tB = tmpB.tile([128, D], f16, tag="tB")
                        pq = [psB.tile([128, DW], f32, tag="psb", name=f"psb{i}")
                              for i in range(ND)]
                        for kc2 in range(KC2):
                            lt = lhsT_B(objT_nc, kc2)
                            for q in range(ND):
                                nc.tensor.matmul(
                                    pq[q],
                                    lhsT=lt,
                                    rhs=wo[:, kc2, :, q * DW:(q + 1) * DW],
                                    start=(kc2 == 0), stop=(kc2 == KC2 - 1),
                                    perf_mode=DRMODE)
                            if a_hook is not None:
                                a_hook(kc2)
                        for q in range(ND):
                            nc.scalar.activation(out=tB[:, q * DW:(q + 1) * DW],
                                                 in_=pq[q], func=AF.Tanh,
                                                 scale=descale)
                        layer_norm_to(tB, 128, oe_nat[:, nch, :])
                        emit_transpose(nch)

                    def emit_window_C(w):
                        """Adjacency + exp for window w (chunks 2w, 2w+1)."""
                        wt = win_tiles.pop(w)
                        padj = psC.tile([F, 256], f32, tag="padj")
                        for kc in range(KC):
                            nc.tensor.matmul(
                                padj,
                                lhsT=veT[:, kc, :],
                                rhs=wt[:, :, kc, :],
                                start=(kc == 0), stop=(kc == KC - 1))
                        # Unnormalized softmax weights: logits are O(1)-bounded
                        # so exp without max-subtraction is safe; accum_out
                        # collects this window's exp-sum for free.
                        e16 = ewp.tile([F, 256], f16, tag="e16")
                        nc.scalar.activation(out=e16, in_=padj, func=AF.Exp,
                                             scale=inv_sqrt_d,
                                             accum_out=psum_w[:, w:w + 1])
                        en = ewp.tile([128, 2, F], f16, tag="en", bufs=2)
                        # [64, 256] -> rows n: [nw, j, f]
                        nc.sync.dma_start(out=en, in_=e16, transpose=True)
                        if DEBUG:
                            nc.sync.dma_start(out=de16_d[w], in_=e16)
                        en_tiles[w] = en

                    def emit_window_D(w):
                        """Aggregation matmuls for window w into split ps_agg."""
                        en = en_tiles.pop(w)
                        for j in range(2):
                            for h in range(2):
                                for u in range(2):
                                    nc.tensor.matmul(
                                        ps_agg[h * F:(h + 1) * F,
                                               u * DW:(u + 1) * DW],
                                        lhsT=en[:, j, :],
                                        rhs=oe_nat[:, 2 * w + j,
                                                   h * DH + u * DW:
                                                   h * DH + (u + 1) * DW],
                                        start=(w == 0 and j == 0), stop=False)

                    # ---- chunks 2-5 with visual branch (A) interleaved ----
                    # A output is partition-split [128, 1024] (d-halves at
                    # base partitions 0/64) so psA is 2 banks, fitting next
                    # to the 5-buf B quarter pool.  One A k-slice rides every
                    # other kc2 slot, pacing wv's fp16 HBM stream (8.4MB)
                    # across four chunks instead of two.
                    with tc.tile_pool(name="wv", bufs=6) as wvp, \
                         tc.tile_pool(name="vt", bufs=1) as vtp, \
                         tc.tile_pool(name="psA", bufs=1, space="PSUM") as psA, \
                         tc.tile_pool(name="tmpA", bufs=1) as tmpA:
                        vt = vtp.tile([128, KC, F], f16)
                        nc.scalar.dma_start(out=vt, in_=vt_d)

                        wv_slices = []

                        def load_wv():
                            kc = len(wv_slices)
                            wv_k = wvp.tile([128, D], f16, tag="wvk")
                            nc.gpsimd.dma_start(out=wv_k,
                                                in_=wv_d[:, kc * D:(kc + 1) * D])
                            wv_slices.append(wv_k)

                        for _ in range(3):
                            load_wv()

                        ps_ve = psA.tile([128, DH], f32)

                        def a_hook_for(nch):
                            def hook(kc2):
                                if kc2 % 2:
                                    return
                                akc = (nch - 2) * (KC2 // 2) + kc2 // 2
                                for h in range(2):
                                    for u in range(2):
                                        nc.tensor.matmul(
                                            ps_ve[h * F:(h + 1) * F,
                                                  u * DW:(u + 1) * DW],
                                            lhsT=vt[:, akc, :],
                                            rhs=wv_slices[akc][
                                                :, h * DH + u * DW:
                                                h * DH + (u + 1) * DW],
                                            start=(akc == 0),
                                            stop=(akc == KC - 1))
                                if len(wv_slices) < KC:
                                    load_wv()
                            return hook

                        for nch in range(2, 6):
                            emit_chunk_B(nch, a_hook=a_hook_for(nch))

                        # ---- phase A epilogue -----------------------------
                        # tanh the split halves; reassemble with one
                        # cross-partition DMA; LN in natural layout.
                        tA = tmpA.tile([F, D], f32)
                        tA2 = tmpA.tile([128, DH], f32)
                        nc.scalar.activation(out=tA2[F:, :], in_=ps_ve[F:, :],
                                             func=AF.Tanh)
                        nc.scalar.dma_start(out=tA[:, DH:], in_=tA2[F:, :])
                        nc.scalar.activation(out=tA[:, :DH], in_=ps_ve[:F, :],
                                             func=AF.Tanh)
                        layer_norm_to(tA, F, ve_nat)
                        ve_bf = tmpB.tile([F, D], f16, tag="tB")
                        nc.vector.tensor_copy(out=ve_bf, in_=ve_nat)
                        # [64, 2048] -> rows d=(kc*128+kl): [kl, kc, f]
                        nc.sync.dma_start(out=veT, in_=ve_bf, transpose=True)

                    # ---- object chunks 6-17 with fused C/D ----------------
                    with tc.tile_pool(name="psD", bufs=1, space="PSUM") as psD:
                        ps_agg = psD.tile([128, DH], f32)

                        fin = {}

                        def final_C_half(h):
                            """Adjacency half h of the final window: matmuls
                            + exp + en transpose."""
                            if h == 0:
                                fin["wt"] = win_tiles.pop(NW - 1)
                                fin["padj"] = psC.tile([F, 256], f32, tag="padj",
                                                       name="padj")
                                fin["e16"] = ewp.tile([F, 256], f16, tag="e16",
                                                      name="e16")
                                fin["en"] = ewp.tile([128, 2, F], f16, tag="en",
                                                     bufs=2, name="en")
                            padj, e16, en = fin["padj"], fin["e16"], fin["en"]
                            for kc in range(KC):
                                nc.tensor.matmul(
                                    padj[:, h * 128:(h + 1) * 128],
                                    lhsT=veT[:, kc, :],
                                    rhs=fin["wt"][:, h:h + 1, kc, :],
                                    start=(kc == 0), stop=(kc == KC - 1))
                            nc.scalar.activation(
                                out=e16[:, h * 128:(h + 1) * 128],
                                in_=padj[:, h * 128:(h + 1) * 128],
                                func=AF.Exp, scale=inv_sqrt_d,
                                accum_out=psum_w[:, NW - 1 + h:NW + h])
                            nc.sync.dma_start(out=en[:, h, :],
                                              in_=e16[:, h * 128:(h + 1) * 128],
                                              transpose=True)

                        for nch in range(6, NCH):
                            emit_chunk_B(nch)
                            for kind, w in sched.get(nch, []):
                                (emit_window_C if kind == "C" else emit_window_D)(w)

                        # ---- drain: final window in halves ----------------
                        final_C_half(0)
                        emit_window_D(NW - 2)
                        final_C_half(1)
                        en = fin["en"]
                        # global softmax denominator (cols 0..NW) -> 1/sum
                        nc.vector.reduce_sum(out=psum_w[:, NW + 1:NW + 2],
                                             in_=psum_w[:, :NW + 1], axis=AX.X)
                        nc.vector.reciprocal(out=psum_w[:, NW + 1:NW + 2],
                                             in_=psum_w[:, NW + 1:NW + 2])
                        # final D window: h=1 half first so its raw merge DMA
                        # overlaps the h=0 matmuls and rescale.
                        tD = tc_win.tile([F, D], f32, tag="winT")
                        for j in range(2):
                            for u in range(2):
                                nc.tensor.matmul(
                                    ps_agg[F:, u * DW:(u + 1) * DW],
                                    lhsT=en[:, j, :],
                                    rhs=oe_nat[:, 2 * NW - 2 + j,
                                               DH + u * DW:DH + (u + 1) * DW],
                                    start=False, stop=(j == 1))
                        # DMA cannot read PSUM: bounce h1 through SBUF
                        drn = tmpB.tile([128, DH], f32, tag="drn", bufs=1)
                        nc.vector.tensor_copy(out=drn[F:, :], in_=ps_agg[F:, :])
                        nc.scalar.dma_start(out=tD[:, DH:], in_=drn[F:, :])
                        for j in range(2):
                            for u in range(2):
                                nc.tensor.matmul(
                                    ps_agg[:F, u * DW:(u + 1) * DW],
                                    lhsT=en[:, j, :],
                                    rhs=oe_nat[:, 2 * NW - 2 + j,
                                               u * DW:(u + 1) * DW],
                                    start=False, stop=(j == 1))
                        # rescale by global 1/sum + add ve (both halves now on
                        # partitions 0-63), tanh, final LN
                        nc.vector.scalar_tensor_tensor(
                            out=tD[:, :DH], in0=ps_agg[:F, :],
                            scalar=psum_w[:, NW + 1:NW + 2],
                            in1=ve_nat[:, :DH], op0=OP.mult, op1=OP.add)
                        nc.vector.scalar_tensor_tensor(
                            out=tD[:, DH:], in0=tD[:, DH:],
                            scalar=psum_w[:, NW + 1:NW + 2],
                            in1=ve_nat[:, DH:], op0=OP.mult, op1=OP.add)
                        nc.scalar.activation(out=tD, in_=tD, func=AF.Tanh)
                        st_f = stats_pool.tile([128, ND, nc.vector.BN_STATS_DIM],
                                               f32, tag="st")
                        for dd in range(ND):
                            nc.vector.bn_stats(out=st_f[:F, dd, :],
                                               in_=tD[:, dd * DW:(dd + 1) * DW])
                        mvr_f = stats_pool.tile([128, 5], f32, tag="mvr")
                        nc.vector.bn_aggr(out=mvr_f[:F, 0:2], in_=st_f[:F])
                        ln_rsqrt(mvr_f, F)
                        # final apply + store in halves on separate queues
                        out_f = tc_win.tile([F, D], f32, tag="winT")
                        for h, eng in ((0, nc.sync), (1, nc.scalar)):
                            nc.vector.tensor_scalar(
                                out=out_f[:, h * DH:(h + 1) * DH],
                                in0=tD[:, h * DH:(h + 1) * DH],
                                scalar1=mvr_f[:F, 0:1], scalar2=mvr_f[:F, 2:3],
                                op0=OP.subtract, op1=OP.mult)
                            eng.dma_start(out=out_d[:, h * DH:(h + 1) * DH],
                                          in_=out_f[:, h * DH:(h + 1) * DH])
                        if DEBUG:
                            nc.sync.dma_start(out=dve_d, in_=ve_nat)
                            nc.sync.dma_start(out=dpw_d, in_=psum_w)
                            nc.sync.dma_start(out=doe_d, in_=oe_nat)

    nc.compile()
    _BUILD_CACHE["nc"] = nc
    return nc


def _numpy_fallback(inputs):
    """Exact fp32 implementation for non-trivial bias/gain fills."""
    def ln(x, g, b, eps=LN_EPS):
        mu = x.mean(-1, keepdims=True)
        var = x.var(-1, keepdims=True)
        return (x - mu) / np.sqrt(var + eps) * g + b

    vf = _f32(inputs["visual_feats"])
    of = _f32(inputs["obj_feats"])
    W_v, b_v = _f32(inputs["W_v"]), _f32(inputs["b_v"])
    W_o, b_o = _f32(inputs["W_o"]), _f32(inputs["b_o"])
    out = np.zeros((BS, F, D), np.float32)
    for i in range(BS):
        ve = ln(np.tanh(vf[i] @ W_v + b_v), _f32(inputs["ln_v_g"]), _f32(inputs["ln_v_b"]))
        oe = ln(np.tanh(of[i].reshape(N, D) @ W_o + b_o),
                _f32(inputs["ln_o_g"]), _f32(inputs["ln_o_b"]))
        adj = oe @ ve.T / np.sqrt(D)
        adj = np.exp(adj - adj.max(0, keepdims=True))
        adj /= adj.sum(0, keepdims=True)
        out[i] = ln(np.tanh(adj.T @ oe + ve),
                    _f32(inputs["ln_ov_g"]), _f32(inputs["ln_ov_b"]))
    return out


def _prep_core_inputs(visual, obj_flat, shared):
    """Per-sample layout prep. visual [64,2048] f32, obj_flat [2304,2048] f32.

    objT DoubleRow layout: [NCH, 128(kl), KC2, 2, 128(n)] with element
    (nch, kl, kc2, i, n) = obj[nch*128+n, kc2*256+i*128+kl] * S_X.
    """
    ot = _q8(obj_flat, S_X).reshape(NCH, 128, KC2, 2, 128).transpose(0, 4, 2, 3, 1)
    if PERF_SW:
        ot = _sw_interleave(np.ascontiguousarray(ot))
    m = {
        "objT": np.ascontiguousarray(ot),
        "vT": np.ascontiguousarray(
            _klc_layout(np.ascontiguousarray(visual.T))).astype(F16),
    }
    m.update(shared)
    return m


def run_kernel(inputs, trace=False):
    """Returns (out [8, 64, 2048] fp32, exec_time_ns or None)."""
    from concourse import bass_utils

    vecs = {k: _f32(inputs[k]) for k in
            ["b_v", "b_o", "ln_v_b", "ln_o_b", "ln_ov_b"]}
    gains = {k: _f32(inputs[k]) for k in ["ln_v_g", "ln_o_g", "ln_ov_g"]}
    trivial = (all(np.all(v == 0) for v in vecs.values())
               and all(np.all(g == 1) for g in gains.values()))
    if not trivial:
        return _numpy_fallback(inputs), None

    visual = _f32(inputs["visual_feats"])            # [8, 64, 2048]
    obj = _f32(inputs["obj_feats"])                  # [8, 64, 36, 2048]
    W_v = _f32(inputs["W_v"])
    W_o = _f32(inputs["W_o"])

    nc = _build()

    # Wo fp8 layout: [128(kl), KC2, 2, D] with (kl, kc2, i, m) =
    # W_o[kc2*256+i*128+kl, m] * S_W.
    wo8 = np.ascontiguousarray(
        _q8(W_o, S_W).reshape(KC2, 2, 128, D).transpose(2, 0, 1, 3))
    shared = {
        "Wo": wo8,
        "Wv": np.ascontiguousarray(_klc_layout(W_v)).astype(F16),
    }
    in_maps = [
        _prep_core_inputs(visual[c], obj[c].reshape(N, D), shared)
        for c in range(BS)
    ]

    res = bass_utils.run_bass_kernel_spmd(
        nc, in_maps, core_ids=list(range(BS)), trace=trace)
    out = np.stack([res.results[c]["out"] for c in range(BS)], axis=0)
    return out.astype(np.float32), res.exec_time_ns


def kernel(**inputs):
    out, _ = run_kernel(inputs, trace=False)
    return out


# revision 23
# speedup vs baseline: 1.5067x; 1.0192x over previous
"""Trainium2 Bass kernel for nn_EnhancedObj (gnn_message_passing).

Per batch sample (data-parallel over 8 cores, one sample per core):
    ve  = LN(tanh(visual @ W_v + b_v))                  [64, 2048]
    oe  = LN(tanh(obj_flat @ W_o + b_o))                [2304, 2048]
    adj = softmax_n(oe @ ve^T / sqrt(2048))             [2304, 64]
    out = LN(tanh(adj^T @ oe + ve))                     [64, 2048]

The object-branch matmul (92% of the FLOPs) runs in fp8-e4m3 with
perf_mode=DoubleRow: the PE virtualizes to 128x256, doing 2 fp8 MACs
per cell per cycle.  Inputs are pre-scaled on the host (obj x32,
W_o x1024) to sit in e4m3's sweet spot; the descale (1/32768) is
folded into the Tanh activation's scale.  Simulated end-to-end error
of this quantization is ~2e-3 vs the 2e-2 gate.  The visual branch
must stay fp16 (fp8 there costs 4.6e-2 — ve feeds the output
directly), and C/D stay fp16 too (fp8 DMA-transpose is unsupported,
so oe would need an extra full vector pass to duplicate in fp8).

B-matmul schedule: per 128-row object chunk, kc2-OUTER loops (8
double-k slices of 256) over 4 output quarters, so each DoubleRow
LDWEIGHTS (256 cols, no FWL, ~0.2us) is amortized over 4 matmuls.
Each chunk holds 4 PSUM quarter-banks; a 5-buf quarter pool lets the
next chunk's first quarter start while the previous chunk's last
quarter drains through Tanh.  Chunks 0-1 run kc2-outer ACROSS both
chunks (8 banks) to track the W_o stream as it lands.

The visual-branch (A) and aggregation (D) outputs are partition-split
[128, 1024] (d-halves at base partitions 0 and 64) instead of
[64, 2048], halving their PSUM footprint (2 banks each) so phase 2
fits: 5 B-quarters + C (0.5) + D (2) = 7.5 of 8 banks.  The halves
are reassembled with one small cross-partition DMA each.

oe transposes for the adjacency ride the otherwise-idle GPSIMD DMA
queue.  Softmax uses unnormalized exp weights (logits O(1)-bounded),
rescaled by the global 1/sum at the end.  LayerNorm's rsqrt is a
vector-engine bit-hack + Newton step, keeping the scalar engine
exclusively on Tanh/Exp (one activation table, no table swaps).

The device kernel assumes the spec's deterministic fills (zero biases,
unit gains).  If non-trivial bias/gain vectors are ever passed, we
fall back to an exact fp32 numpy implementation.
"""

import numpy as np
import ml_dtypes

F16 = np.float16
F8 = ml_dtypes.float8_e4m3

BS = 8          # batch (== number of cores)
F = 64          # win_len (frames)
OBJ = 36        # objects per frame
D = 2048        # feature dim
N = F * OBJ     # 2304 objects per sample
NCH = N // 128  # 18 object-row chunks
NW = NCH // 2   # 9 two-chunk adjacency windows
KC = D // 128   # 16 contraction chunks (fp16 matmuls)
KC2 = D // 256  # 8 double-k chunks (fp8 DoubleRow)
DW = 512        # B-matmul quarter width (one PSUM bank of fp32)
ND = D // DW    # 4 output-column groups
DH = D // 2     # 1024: partition-split half width for A and D
LN_EPS = 1e-5
RSQRT_MAGIC = 0x5F3759DF

S_X = 32.0      # obj fp8 pre-scale
S_W = 1024.0    # W_o fp8 pre-scale
PERF_SW = False  # DoubleRow (3D AP) vs DoubleRowSwInterleave layout
DEBUG = False    # add per-stage debug outputs

_BUILD_CACHE = {}


def _f32(x):
    return np.ascontiguousarray(np.asarray(x), dtype=np.float32)


def _q8(x, s):
    return np.clip(np.asarray(x, np.float32) * s, -240.0, 240.0).astype(F8)


def _klc_layout(w):
    """[D, M] -> [128(kl), KC*M] with element (kl, kc, m) = w[kc*128+kl, m]."""
    d, m = w.shape
    assert d == D
    return w.reshape(KC, 128, m).transpose(1, 0, 2).reshape(128, KC * m)


def _sw_interleave(a):
    """[..., 2, 128] DoubleRow pairs -> [..., 256] SwInterleave layout.

    Logical stationary element (i, m) goes to free position 2*(127-m)+i.
    """
    out = np.empty(a.shape[:-2] + (256,), dtype=a.dtype)
    out[..., 0::2] = a[..., 0, ::-1]
    out[..., 1::2] = a[..., 1, ::-1]
    return out


def _build():
    """Build + compile the SPMD Bass program (trivial-fill fast path)."""
    if "nc" in _BUILD_CACHE:
        return _BUILD_CACHE["nc"]

    import concourse.bacc as bacc
    import concourse.tile as tile
    from concourse import mybir

    f32 = mybir.dt.float32
    f16 = mybir.dt.float16
    f8 = mybir.dt.float8e4
    i32 = mybir.dt.int32
    AF = mybir.ActivationFunctionType
    AX = mybir.AxisListType
    OP = mybir.AluOpType
    DRMODE = (mybir.MatmulPerfMode.DoubleRowSwInterleave if PERF_SW
              else mybir.MatmulPerfMode.DoubleRow)

    nc = bacc.Bacc("TRN2", target_bir_lowering=False, debug=False, num_devices=BS)

    objT_shape = ([NCH, 128, KC2, 256] if PERF_SW
                  else [NCH, 128, KC2, 2, 128])
    objT_d = nc.dram_tensor("objT", objT_shape, f8, kind="ExternalInput").ap()
    wo_d = nc.dram_tensor("Wo", [128, KC2, 2, D], f8, kind="ExternalInput").ap()
    wv_d = nc.dram_tensor("Wv", [128, KC * D], f16, kind="ExternalInput").ap()
    vt_d = nc.dram_tensor("vT", [128, KC * F], f16, kind="ExternalInput").ap()
    out_d = nc.dram_tensor("out", [F, D], f32, kind="ExternalOutput").ap()
    if DEBUG:
        dve_d = nc.dram_tensor("dve", [F, D], f32, kind="ExternalOutput").ap()
        de16_d = nc.dram_tensor("de16", [NW, F, 256], f16, kind="ExternalOutput").ap()
        dpw_d = nc.dram_tensor("dpw", [F, NW + 3], f32, kind="ExternalOutput").ap()
        doe_d = nc.dram_tensor("doe", [128, NCH, D], f16, kind="ExternalOutput").ap()

    inv_sqrt_d = 1.0 / float(np.sqrt(D))
    descale = 1.0 / (S_X * S_W)

    # adjacency (C) / aggregation (D) emission points: window w covers
    # object chunks (2w, 2w+1); C(w) needs both transposed + veT (ready
    # ~chunk 4); D(w) follows C(w) one chunk later.  Window NW-1 drains
    # manually after the loop.
    sched = {}
    for w in range(NW):
        c_at = 7 + w if w < 3 else max(2 * w + 3, 10)
        c_at = min(c_at, NCH - 1) if w < NW - 1 else NCH
        d_at = c_at + 1
        if c_at < NCH:
            sched.setdefault(c_at, []).append(("C", w))
        if d_at < NCH:
            sched.setdefault(d_at, []).append(("D", w))

    def lhsT_B(objT_tile, kc2):
        if PERF_SW:
            return objT_tile[:, kc2, :]
        return objT_tile[:, kc2, :, :]

    with tile.TileContext(nc) as tc:
        with tc.tile_pool(name="persist", bufs=1) as persist, \
             tc.tile_pool(name="stats", bufs=2) as stats_pool:

            def ln_rsqrt(mvr, rows):
                """mvr[:,1]=var -> mvr[:,2]=1/sqrt(var+eps), vector engine
                only (bit-hack seed + 1 Newton step; no act-table load)."""
                x, y, t = mvr[:rows, 3:4], mvr[:rows, 2:3], mvr[:rows, 4:5]
                nc.vector.tensor_scalar(out=x, in0=mvr[:rows, 1:2],
                                        scalar1=LN_EPS, scalar2=None, op0=OP.add)
                nc.vector.tensor_scalar(out=t.bitcast(i32), in0=x.bitcast(i32),
                                        scalar1=1, scalar2=None,
                                        op0=OP.logical_shift_right)
                # magic - (i>>1)  ==  (i>>1) * -1 + magic  (both ops arith)
                nc.vector.tensor_scalar(out=y.bitcast(i32), in0=t.bitcast(i32),
                                        scalar1=-1, scalar2=RSQRT_MAGIC,
                                        op0=OP.mult, op1=OP.add)
                nc.vector.tensor_tensor(out=t, in0=y, in1=y, op=OP.mult)
                nc.vector.tensor_tensor(out=t, in0=t, in1=x, op=OP.mult)
                nc.vector.tensor_scalar(out=t, in0=t, scalar1=-0.5,
                                        scalar2=1.5, op0=OP.mult, op1=OP.add)
                nc.vector.tensor_tensor(out=y, in0=y, in1=t, op=OP.mult)

            def layer_norm_to(t_in, rows, out_tile):
                """LN over the free dim of t_in[:rows] -> out_tile (casts)."""
                st = stats_pool.tile([128, ND, nc.vector.BN_STATS_DIM], f32, tag="st")
                for j in range(ND):
                    nc.vector.bn_stats(out=st[:rows, j, :],
                                       in_=t_in[:rows, j * DW:(j + 1) * DW])
                mvr = stats_pool.tile([128, 5], f32, tag="mvr")
                nc.vector.bn_aggr(out=mvr[:rows, 0:2], in_=st[:rows])
                ln_rsqrt(mvr, rows)
                nc.vector.tensor_scalar(
                    out=out_tile[:rows], in0=t_in[:rows],
                    scalar1=mvr[:rows, 0:1], scalar2=mvr[:rows, 2:3],
                    op0=OP.subtract, op1=OP.mult)

            ve_nat = persist.tile([F, D], f32)          # LN'd visual embedding
            veT = persist.tile([128, KC, F], f16)       # transposed, for adjacency
            oe_nat = persist.tile([128, NCH, D], f16)   # LN'd object embeddings
            psum_w = persist.tile([F, NW + 3], f32)     # per-window exp sums

            with tc.tile_pool(name="wo", bufs=1) as wop, \
                 tc.tile_pool(name="objs", bufs=4) as objp, \
                 tc.tile_pool(name="win", bufs=4) as tc_win, \
                 tc.tile_pool(name="ew", bufs=1) as ewp:
                wo = wop.tile([128, KC2, 2, D], f8)

                obj_tiles = {}
                win_tiles = {}
                en_tiles = {}

                # objT (and later wv) ride the otherwise-idle GPSIMD DMA
                # queue; the sync queue carries W_o + the oe transposes.
                def load_objT(nch):
                    t = objp.tile(objT_shape[1:], f8, name="objT", tag="objT")
                    nc.gpsimd.dma_start(out=t, in_=objT_d[nch])
                    obj_tiles[nch] = t

                def win_tiles_tile(nch):
                    w = nch // 2
                    if w not in win_tiles:
                        win_tiles[w] = tc_win.tile([128, 2, KC, 128], f16,
                                                   name="winT", tag="winT")
                    return win_tiles[w][:, nch % 2, :, :]

                def emit_transpose(nch):
                    nc.sync.dma_start(out=win_tiles_tile(nch),
                                      in_=oe_nat[:, nch, :], transpose=True)

                load_objT(0)
                load_objT(1)
                for kc2 in range(KC2):
                    for mh in range(2):
                        nc.sync.dma_start(out=wo[:, kc2, :, mh * DH:(mh + 1) * DH],
                                          in_=wo_d[:, kc2, :, mh * DH:(mh + 1) * DH])

                # ---- chunks 0-1: kc2-outer across both chunks ----------
                # 8 PSUM quarter-banks live so each arriving W_o k-slice
                # feeds 8 DoubleRow matmuls: the PE tracks the W_o stream.
                with tc.tile_pool(name="ps01", bufs=1, space="PSUM") as ps01, \
                     tc.tile_pool(name="t01", bufs=2) as t01p:
                    pq01 = {}
                    for c in range(2):
                        for q in range(ND):
                            pq01[c, q] = ps01.tile([128, DW], f32,
                                                   tag=f"q{c}{q}", name=f"pq{c}{q}")
                    for kc2 in range(KC2):
                        for mh in range(2):
                            for c in range(2):
                                lt = lhsT_B(obj_tiles[c], kc2)
                                for q in (2 * mh, 2 * mh + 1):
                                    nc.tensor.matmul(
                                        pq01[c, q],
                                        lhsT=lt,
                                        rhs=wo[:, kc2, :, q * DW:(q + 1) * DW],
                                        start=(kc2 == 0), stop=(kc2 == KC2 - 1),
                                        perf_mode=DRMODE)
                    for c in range(2):
                        obj_tiles.pop(c)
                        tB01 = t01p.tile([128, D], f16, tag="tB01", name=f"t01_{c}")
                        for q in range(ND):
                            nc.scalar.activation(out=tB01[:, q * DW:(q + 1) * DW],
                                                 in_=pq01[c, q], func=AF.Tanh,
                                                 scale=descale)
                        layer_norm_to(tB01, 128, oe_nat[:, c, :])
                        emit_transpose(c)
                    for c in (2, 3):
                        t = objp.tile(objT_shape[1:], f8, name="objT", tag="objT")
                        nc.sync.dma_start(out=t, in_=objT_d[c])
                        obj_tiles[c] = t

                with tc.tile_pool(name="psB", bufs=5, space="PSUM") as psB, \
                     tc.tile_pool(name="psC", bufs=1, space="PSUM") as psC, \
                     tc.tile_pool(name="tmpB", bufs=2) as tmpB:

                    def emit_chunk_B(nch, a_hook=None):
                        objT_nc = obj_tiles.pop(nch)
                        if nch + 2 < NCH:
                            load_objT(nch + 2)
                        tB = tmpB.tile([128, D], f16, tag="tB")
                        pq = [psB.tile([128, DW], f32, tag="psb", name=f"psb{i}")
                              for i in range(ND)]
                        for kc2 in range(KC2):
                            lt = lhsT_B(objT_nc, kc2)
                            for q in range(ND):
                                nc.tensor.matmul(
                                    pq[q],
                                    lhsT=lt,
                                    rhs=wo[:, kc2, :, q * DW:(q + 1) * DW],
                                    start=(kc2 == 0), stop=(kc2 == KC2 - 1),
                                    perf_mode=DRMODE)
                                if kc2 == KC2 - 1:
                                    # tanh each quarter the moment it stops, so
                                    # its PSUM bank frees before the next
                                    # chunk's rotation needs it
                                    nc.scalar.activation(
                                        out=tB[:, q * DW:(q + 1) * DW],
                                        in_=pq[q], func=AF.Tanh, scale=descale)
                            if a_hook is not None:
                                a_hook(kc2)
                        layer_norm_to(tB, 128, oe_nat[:, nch, :])
                        emit_transpose(nch)

                    def emit_window_C(w):
                        """Adjacency + exp for window w (chunks 2w, 2w+1)."""
                        wt = win_tiles.pop(w)
                        padj = psC.tile([F, 256], f32, tag="padj")
                        for kc in range(KC):
                            nc.tensor.matmul(
                                padj,
                                lhsT=veT[:, kc, :],
                                rhs=wt[:, :, kc, :],
                                start=(kc == 0), stop=(kc == KC - 1))
                        # Unnormalized softmax weights: logits are O(1)-bounded
                        # so exp without max-subtraction is safe; accum_out
                        # collects this window's exp-sum for free.
                        e16 = ewp.tile([F, 256], f16, tag="e16")
                        nc.scalar.activation(out=e16, in_=padj, func=AF.Exp,
                                             scale=inv_sqrt_d,
                                             accum_out=psum_w[:, w:w + 1])
                        en = ewp.tile([128, 2, F], f16, tag="en", bufs=2)
                        # [64, 256] -> rows n: [nw, j, f]
                        nc.sync.dma_start(out=en, in_=e16, transpose=True)
                        if DEBUG:
                            nc.sync.dma_start(out=de16_d[w], in_=e16)
                        en_tiles[w] = en

                    def emit_window_D(w):
                        """Aggregation matmuls for window w into split ps_agg."""
                        en = en_tiles.pop(w)
                        for j in range(2):
                            for h in range(2):
                                for u in range(2):
                                    nc.tensor.matmul(
                                        ps_agg[h * F:(h + 1) * F,
                                               u * DW:(u + 1) * DW],
                                        lhsT=en[:, j, :],
                                        rhs=oe_nat[:, 2 * w + j,
                                                   h * DH + u * DW:
                                                   h * DH + (u + 1) * DW],
                                        start=(w == 0 and j == 0), stop=False)

                    # ---- chunks 2-5 with visual branch (A) interleaved ----
                    # A output is partition-split [128, 1024] (d-halves at
                    # base partitions 0/64) so psA is 2 banks, fitting next
                    # to the 5-buf B quarter pool.  One A k-slice rides every
                    # other kc2 slot, pacing wv's fp16 HBM stream (8.4MB)
                    # across four chunks instead of two.
                    with tc.tile_pool(name="wv", bufs=6) as wvp, \
                         tc.tile_pool(name="vt", bufs=1) as vtp, \
                         tc.tile_pool(name="psA", bufs=1, space="PSUM") as psA, \
                         tc.tile_pool(name="tmpA", bufs=1) as tmpA:
                        vt = vtp.tile([128, KC, F], f16)
                        nc.gpsimd.dma_start(out=vt, in_=vt_d)

                        wv_slices = []

                        def load_wv(eng=None):
                            kc = len(wv_slices)
                            wv_k = wvp.tile([128, D], f16, tag="wvk")
                            (eng or nc.gpsimd).dma_start(
                                out=wv_k, in_=wv_d[:, kc * D:(kc + 1) * D])
                            wv_slices.append(wv_k)

                        load_wv()            # wv0 early on gpsimd
                        load_wv(eng=nc.sync)  # wv1, wv2 behind the wo stream
                        load_wv(eng=nc.sync)

                        ps_ve = psA.tile([128, DH], f32)

                        def a_hook_for(nch):
                            def hook(kc2):
                                if kc2 % 2:
                                    return
                                akc = (nch - 2) * (KC2 // 2) + kc2 // 2
                                for h in range(2):
                                    for u in range(2):
                                        nc.tensor.matmul(
                                            ps_ve[h * F:(h + 1) * F,
                                                  u * DW:(u + 1) * DW],
                                            lhsT=vt[:, akc, :],
                                            rhs=wv_slices[akc][
                                                :, h * DH + u * DW:
                                                h * DH + (u + 1) * DW],
                                            start=(akc == 0),
                                            stop=(akc == KC - 1))
                                if len(wv_slices) < KC:
                                    load_wv()
                            return hook

                        for nch in range(2, 6):
                            emit_chunk_B(nch, a_hook=a_hook_for(nch))

                        # ---- phase A epilogue -----------------------------
                        # tanh the split halves; reassemble with one
                        # cross-partition DMA; LN in natural layout.
                        tA = tmpA.tile([F, D], f32)
                        tA2 = tmpA.tile([128, DH], f32)
                        nc.scalar.activation(out=tA2[F:, :], in_=ps_ve[F:, :],
                                             func=AF.Tanh)
                        nc.scalar.dma_start(out=tA[:, DH:], in_=tA2[F:, :])
                        nc.scalar.activation(out=tA[:, :DH], in_=ps_ve[:F, :],
                                             func=AF.Tanh)
                        layer_norm_to(tA, F, ve_nat)
                        ve_bf = tmpB.tile([F, D], f16, tag="tB")
                        nc.vector.tensor_copy(out=ve_bf, in_=ve_nat)
                        # [64, 2048] -> rows d=(kc*128+kl): [kl, kc, f]
                        nc.sync.dma_start(out=veT, in_=ve_bf, transpose=True)

                    # ---- object chunks 6-17 with fused C/D ----------------
                    with tc.tile_pool(name="psD", bufs=1, space="PSUM") as psD:
                        ps_agg = psD.tile([128, DH], f32)

                        fin = {}

                        def final_C_half(h):
                            """Adjacency half h of the final window: matmuls
                            + exp + en transpose."""
                            if h == 0:
                                fin["wt"] = win_tiles.pop(NW - 1)
                                fin["padj"] = psC.tile([F, 256], f32, tag="padj",
                                                       name="padj")
                                fin["e16"] = ewp.tile([F, 256], f16, tag="e16",
                                                      name="e16")
                                fin["en"] = ewp.tile([128, 2, F], f16, tag="en",
                                                     bufs=2, name="en")
                            padj, e16, en = fin["padj"], fin["e16"], fin["en"]
                            for kc in range(KC):
                                nc.tensor.matmul(
                                    padj[:, h * 128:(h + 1) * 128],
                                    lhsT=veT[:, kc, :],
                                    rhs=fin["wt"][:, h:h + 1, kc, :],
                                    start=(kc == 0), stop=(kc == KC - 1))
                            nc.scalar.activation(
                                out=e16[:, h * 128:(h + 1) * 128],
                                in_=padj[:, h * 128:(h + 1) * 128],
                                func=AF.Exp, scale=inv_sqrt_d,
                                accum_out=psum_w[:, NW - 1 + h:NW + h])
                            nc.sync.dma_start(out=en[:, h, :],
                                              in_=e16[:, h * 128:(h + 1) * 128],
                                              transpose=True)

                        for nch in range(6, NCH):
                            emit_chunk_B(nch)
                            for kind, w in sched.get(nch, []):
                                (emit_window_C if kind == "C" else emit_window_D)(w)

                        # ---- drain: final window in halves ----------------
                        final_C_half(0)
                        emit_window_D(NW - 2)
                        final_C_half(1)
                        en = fin["en"]
                        # global softmax denominator (cols 0..NW) -> 1/sum
                        nc.vector.reduce_sum(out=psum_w[:, NW + 1:NW + 2],
                                             in_=psum_w[:, :NW + 1], axis=AX.X)
                        nc.vector.reciprocal(out=psum_w[:, NW + 1:NW + 2],
                                             in_=psum_w[:, NW + 1:NW + 2])
                        # final D window: h=0 half first — its rescale can
                        # start from PSUM directly while h=1's raw merge DMA
                        # (vector bounce + SBUF DMA) is still in flight.
                        tD = tc_win.tile([F, D], f32, tag="winT")
                        for j in range(2):
                            for u in range(2):
                                nc.tensor.matmul(
                                    ps_agg[:F, u * DW:(u + 1) * DW],
                                    lhsT=en[:, j, :],
                                    rhs=oe_nat[:, 2 * NW - 2 + j,
                                               u * DW:(u + 1) * DW],
                                    start=False, stop=(j == 1))
                        for j in range(2):
                            for u in range(2):
                                nc.tensor.matmul(
                                    ps_agg[F:, u * DW:(u + 1) * DW],
                                    lhsT=en[:, j, :],
                                    rhs=oe_nat[:, 2 * NW - 2 + j,
                                               DH + u * DW:DH + (u + 1) * DW],
                                    start=False, stop=(j == 1))
                        # DMA cannot read PSUM: bounce h1 through SBUF
                        drn = tmpB.tile([128, DH], f32, tag="drn", bufs=1)
                        nc.vector.tensor_copy(out=drn[F:, :], in_=ps_agg[F:, :])
                        nc.scalar.dma_start(out=tD[:, DH:], in_=drn[F:, :])
                        # rescale by global 1/sum + add ve, tanh, stats —
                        # pipelined per 512-wide quarter across vector/scalar
                        st_f = stats_pool.tile([128, ND, nc.vector.BN_STATS_DIM],
                                               f32, tag="st")
                        for dd in range(ND):
                            sl = slice(dd * DW, (dd + 1) * DW)
                            src = ps_agg[:F, sl] if dd < 2 else tD[:, sl]
                            nc.vector.scalar_tensor_tensor(
                                out=tD[:, sl], in0=src,
                                scalar=psum_w[:, NW + 1:NW + 2],
                                in1=ve_nat[:, sl], op0=OP.mult, op1=OP.add)
                            nc.scalar.activation(out=tD[:, sl], in_=tD[:, sl],
                                                 func=AF.Tanh)
                            nc.vector.bn_stats(out=st_f[:F, dd, :],
                                               in_=tD[:, sl])
                        mvr_f = stats_pool.tile([128, 5], f32, tag="mvr")
                        nc.vector.bn_aggr(out=mvr_f[:F, 0:2], in_=st_f[:F])
                        ln_rsqrt(mvr_f, F)
                        # final apply + store in halves on separate queues
                        out_f = tc_win.tile([F, D], f32, tag="winT")
                        for h, eng in ((0, nc.sync), (1, nc.scalar)):
                            nc.vector.tensor_scalar(
                                out=out_f[:, h * DH:(h + 1) * DH],
                                in0=tD[:, h * DH:(h + 1) * DH],
                                scalar1=mvr_f[:F, 0:1], scalar2=mvr_f[:F, 2:3],
                                op0=OP.subtract, op1=OP.mult)
                            eng.dma_start(out=out_d[:, h * DH:(h + 1) * DH],
                                          in_=out_f[:, h * DH:(h + 1) * DH])
                        if DEBUG:
                            nc.sync.dma_start(out=dve_d, in_=ve_nat)
                            nc.sync.dma_start(out=dpw_d, in_=psum_w)
                            nc.sync.dma_start(out=doe_d, in_=oe_nat)

    nc.compile()
    _BUILD_CACHE["nc"] = nc
    return nc


def _numpy_fallback(inputs):
    """Exact fp32 implementation for non-trivial bias/gain fills."""
    def ln(x, g, b, eps=LN_EPS):
        mu = x.mean(-1, keepdims=True)
        var = x.var(-1, keepdims=True)
        return (x - mu) / np.sqrt(var + eps) * g + b

    vf = _f32(inputs["visual_feats"])
    of = _f32(inputs["obj_feats"])
    W_v, b_v = _f32(inputs["W_v"]), _f32(inputs["b_v"])
    W_o, b_o = _f32(inputs["W_o"]), _f32(inputs["b_o"])
    out = np.zeros((BS, F, D), np.float32)
    for i in range(BS):
        ve = ln(np.tanh(vf[i] @ W_v + b_v), _f32(inputs["ln_v_g"]), _f32(inputs["ln_v_b"]))
        oe = ln(np.tanh(of[i].reshape(N, D) @ W_o + b_o),
                _f32(inputs["ln_o_g"]), _f32(inputs["ln_o_b"]))
        adj = oe @ ve.T / np.sqrt(D)
        adj = np.exp(adj - adj.max(0, keepdims=True))
        adj /= adj.sum(0, keepdims=True)
        out[i] = ln(np.tanh(adj.T @ oe + ve),
                    _f32(inputs["ln_ov_g"]), _f32(inputs["ln_ov_b"]))
    return out


def _prep_core_inputs(visual, obj_flat, shared):
    """Per-sample layout prep. visual [64,2048] f32, obj_flat [2304,2048] f32.

    objT DoubleRow layout: [NCH, 128(kl), KC2, 2, 128(n)] with element
    (nch, kl, kc2, i, n) = obj[nch*128+n, kc2*256+i*128+kl] * S_X.
    """
    ot = _q8(obj_flat, S_X).reshape(NCH, 128, KC2, 2, 128).transpose(0, 4, 2, 3, 1)
    if PERF_SW:
        ot = _sw_interleave(np.ascontiguousarray(ot))
    m = {
        "objT": np.ascontiguousarray(ot),
        "vT": np.ascontiguousarray(
            _klc_layout(np.ascontiguousarray(visual.T))).astype(F16),
    }
    m.update(shared)
    return m


def run_kernel(inputs, trace=False):
    """Returns (out [8, 64, 2048] fp32, exec_time_ns or None)."""
    from concourse import bass_utils

    vecs = {k: _f32(inputs[k]) for k in
            ["b_v", "b_o", "ln_v_b", "ln_o_b", "ln_ov_b"]}
    gains = {k: _f32(inputs[k]) for k in ["ln_v_g", "ln_o_g", "ln_ov_g"]}
    trivial = (all(np.all(v == 0) for v in vecs.values())
               and all(np.all(g == 1) for g in gains.values()))
    if not trivial:
        return _numpy_fallback(inputs), None

    visual = _f32(inputs["visual_feats"])            # [8, 64, 2048]
    obj = _f32(inputs["obj_feats"])                  # [8, 64, 36, 2048]
    W_v = _f32(inputs["W_v"])
    W_o = _f32(inputs["W_o"])

    nc = _build()

    # Wo fp8 layout: [128(kl), KC2, 2, D] with (kl, kc2, i, m) =
    # W_o[kc2*256+i*128+kl, m] * S_W.
    wo8 = np.ascontiguousarray(
        _q8(W_o, S_W).reshape(KC2, 2, 128, D).transpose(2, 0, 1, 3))
    shared = {
        "Wo": wo8,
        "Wv": np.ascontiguousarray(_klc_layout(W_v)).astype(F16),
    }
    in_maps = [
        _prep_core_inputs(visual[c], obj[c].reshape(N, D), shared)
        for c in range(BS)
    ]

    res = bass_utils.run_bass_kernel_spmd(
        nc, in_maps, core_ids=list(range(BS)), trace=trace)
    out = np.stack([res.results[c]["out"] for c in range(BS)], axis=0)
    return out.astype(np.float32), res.exec_time_ns


def kernel(**inputs):
    out, _ = run_kernel(inputs, trace=False)
    return out
